# revision 1
# baseline (speedup 1.0000x reference)
"""GPT-style transformer forward on 8 Trainium2 NeuronCores.

Sharding: data-parallel over batch (2 groups of 4 cores), tensor-parallel
within each group (heads / FFN hidden / vocab columns split 4 ways).
Device activations are feature-major [feature, token] so all matmuls run
without transposes.

v2: host-reshaped weights so every chunk DMA is contiguous; per-token-block
pipelined collectives (AllGather of attention head outputs, AllReduce of FFN
partials); broadcast-stat layernorm balanced across Vector/GpSimd/Scalar;
SBUF-resident vocab logits with pipelined output writes.
"""

import os
from contextlib import ExitStack

import numpy as np
import ml_dtypes

import concourse.bass as bass
import concourse.mybir as mybir
import concourse.tile as tile
from concourse.bass_utils import run_bass_kernel_spmd
from concourse.vector_clock import ScopedClock


def _drain_and_barrier(self, tick_clock, wait_clock):
    """The walrus build here encodes Drain/NoOp as TPB_CTRL with at most one
    sync-wait slot; Tile's stock tail attaches all outstanding waits to the
    Drain and fails codegen. Split the waits one-per-NOP instead."""
    nop_inst = self.nc.sync.nop(nofuse=True)
    wait_clock.add_sem_waits(nop_inst.ins, ScopedClock({None: tick_clock.global_clock}))
    si = nop_inst.ins.sync_info
    if si is not None and len(si.on_wait) > 1:
        waits = list(si.on_wait)
        nop_inst.ins.sync_info = mybir.SyncInfo(on_wait=waits[:1], on_update=list(si.on_update))
        for w in waits[1:]:
            n2 = self.nc.sync.nop(nofuse=True)
            n2.ins.sync_info = mybir.SyncInfo(on_wait=[w], on_update=[])
    self.nc.sync.drain()
    self.nc.all_engine_barrier()
    assert self.sems is not None
    popped = self.nc._tile_sem_poison_stack.pop()
    assert popped is self._sem_poison
    self.nc.clear_and_free_semaphores(list(self.sems.allocated().values()))
    self.nc.all_engine_barrier()


tile.TileContext._drain_and_barrier = _drain_and_barrier

_MAX_WAITS = 1  # this walrus build caps sync-waits per instruction


def split_sync_waits(nc):
    """Hoist excess on_wait entries onto same-engine NOPs inserted before the
    instruction (engine queues execute in program order, so semantics hold)."""
    n = 0
    for bb in nc.main_func.blocks:
        insts = bb.instructions
        new_list = []
        for inst in insts:
            si = getattr(inst, "sync_info", None)
            if si is not None and len(si.on_wait) > _MAX_WAITS:
                waits = list(si.on_wait)
                for w in waits[:-_MAX_WAITS]:
                    n += 1
                    new_list.append(mybir.InstNoOp(
                        name=f"{inst.name}-sw{n}",
                        sync_info=mybir.SyncInfo(on_wait=[w], on_update=[]),
                        bass_nofuse=True,
                        engine=inst.engine,
                    ))
                inst.sync_info = mybir.SyncInfo(
                    on_wait=waits[-_MAX_WAITS:], on_update=list(si.on_update)
                )
            new_list.append(inst)
        if len(new_list) != len(insts):
            bb.instructions[:] = new_list
    return n


# Model dims (hardcoded per problem spec)
L_FULL, H, D, V, SMAX = 8, 16, 1024, 32000, 1024
DH = D // H          # 64
FF = 4 * D           # 4096
B, S = 2, 1024
T = S                # tokens per group (one batch element per group)
TP = 4               # tensor-parallel degree within a group
HL = H // TP         # 4 local heads
FFL = FF // TP       # 1024 local FFN cols
VL = V // TP         # 8000 local vocab cols
VLP = 8064           # padded to 63*128
NVM = VLP // 128     # 63 vocab m-tiles
EPS = 1e-5
KT = D // 128        # 8 k-tiles over model dim
NB = T // 512        # 2 token blocks of 512

BF = mybir.dt.bfloat16
F32 = mybir.dt.float32
AF = mybir.ActivationFunctionType
ALU = mybir.AluOpType

RG = [[0, 1, 2, 3], [4, 5, 6, 7]]

N_LAYERS = int(os.environ.get("BASS_GPT_LAYERS", str(L_FULL)))
SKIP_FINAL = os.environ.get("BASS_GPT_SKIP_FINAL", "0") == "1"


def _r2(ap):
    """[ (kt p) n ] -> [p kt n] view of a DRAM 2-D tensor (p=128)."""
    return ap.rearrange("(kt p) n -> p kt n", p=128)


def build_program():
    nc = bass.Bass("TRN2")

    # ---- DRAM parameters (per-core shards, host-reshaped) ----
    h0b = nc.declare_dram_parameter("h0b", [D, T], BF, isOutput=False)
    wqkv = nc.declare_dram_parameter("wqkv", [N_LAYERS, D, 3 * HL * DH], BF, isOutput=False)
    bqkv = nc.declare_dram_parameter("bqkv", [N_LAYERS, 3 * HL * DH], F32, isOutput=False)
    wo_f = nc.declare_dram_parameter("wo_f", [N_LAYERS, D, D], BF, isOutput=False)
    bo = nc.declare_dram_parameter("bo", [N_LAYERS, D], F32, isOutput=False)
    ln1g = nc.declare_dram_parameter("ln1g", [N_LAYERS, D], F32, isOutput=False)
    ln1b = nc.declare_dram_parameter("ln1b", [N_LAYERS, D], F32, isOutput=False)
    w1r = nc.declare_dram_parameter("w1r", [N_LAYERS, KT, 128, KT, 128], BF, isOutput=False)
    b1 = nc.declare_dram_parameter("b1", [N_LAYERS, FFL], F32, isOutput=False)
    w2r = nc.declare_dram_parameter("w2r", [N_LAYERS, KT, 128, KT, 128], BF, isOutput=False)
    b2q = nc.declare_dram_parameter("b2q", [N_LAYERS, D], F32, isOutput=False)
    ln2g = nc.declare_dram_parameter("ln2g", [N_LAYERS, D], F32, isOutput=False)
    ln2b = nc.declare_dram_parameter("ln2b", [N_LAYERS, D], F32, isOutput=False)
    woutr = nc.declare_dram_parameter("woutr", [NVM, 128, KT, 128], BF, isOutput=False)
    bout = nc.declare_dram_parameter("bout", [VLP], F32, isOutput=False)
    out = nc.declare_dram_parameter("out", [VLP, T], F32, isOutput=True)

    with ExitStack() as ctx:
        tc = ctx.enter_context(tile.TileContext(nc))

        def _scope(name):
            sid, _ = nc.enter_named_scope(name, False)
            return (name, sid)

        def _escope(h):
            nc.leave_named_scope(h[0], h[1], False)

        # ---- outer pools (live whole program) ----
        const = ctx.enter_context(tc.tile_pool(name="const", bufs=1))
        hpool = ctx.enter_context(tc.tile_pool(name="hpool", bufs=1))
        spool = ctx.enter_context(tc.tile_pool(name="spool", bufs=2))
        rpool = ctx.enter_context(tc.tile_pool(name="rpool", bufs=2))
        mm_psum = ctx.enter_context(tc.tile_pool(name="mm_psum", bufs=3, space="PSUM"))
        bc_psum = ctx.enter_context(tc.tile_pool(name="bc_psum", bufs=2, space="PSUM"))
        dram = ctx.enter_context(tc.tile_pool(name="dram", bufs=2, space="DRAM"))

        # ---- constants ----
        ones_k = const.tile([128, 1], BF)       # lhsT for partition-sum (M=1)
        nc.vector.memset(ones_k, 1.0)
        ones128 = const.tile([128, 128], BF)    # lhsT for bcast partition-sum (M=128)
        nc.vector.memset(ones128, 1.0)
        ones_m = const.tile([1, 128], F32)      # lhsT for broadcast (K=1, M=128)
        nc.vector.memset(ones_m, 1.0)
        eps128 = const.tile([128, 1], F32)
        nc.vector.memset(eps128, float(EPS))
        # causal keep-masks: variant j keeps where t1f - t2p - 128*j >= 0
        maskq = const.tile([128, 4, 512], BF)
        nc.gpsimd.memset(maskq, 1.0)
        for j in range(4):
            nc.gpsimd.affine_select(
                out=maskq[:, j, :], in_=maskq[:, j, :],
                compare_op=ALU.is_ge, fill=0.0,
                base=-128 * j, pattern=[[1, 512]], channel_multiplier=-1,
            )

        # ---- persistent activation state ----
        hb = hpool.tile([128, KT, T], BF)       # residual stream (feature-major)
        nc.sync.dma_start(hb, _r2(h0b))

        with ExitStack() as lctx:
            apool = lctx.enter_context(tc.tile_pool(name="apool", bufs=1))
            xpool = lctx.enter_context(tc.tile_pool(name="xpool", bufs=1))
            fpool = lctx.enter_context(tc.tile_pool(name="fpool", bufs=1))
            epool = lctx.enter_context(tc.tile_pool(name="epool", bufs=2))
            wq_pool = lctx.enter_context(tc.tile_pool(name="wq_pool", bufs=1))
            wo_pool = lctx.enter_context(tc.tile_pool(name="wo_pool", bufs=1))
            wch_pool = lctx.enter_context(tc.tile_pool(name="wch_pool", bufs=4))
            agp = lctx.enter_context(tc.tile_pool(name="agp", bufs=2))
            oarp = lctx.enter_context(tc.tile_pool(name="oarp", bufs=1))
            xsqp = lctx.enter_context(tc.tile_pool(name="xsqp", bufs=4))
            bpool = lctx.enter_context(tc.tile_pool(name="bpool", bufs=2))
            stp = lctx.enter_context(tc.tile_pool(name="stp", bufs=1))
            o_psum = lctx.enter_context(tc.tile_pool(name="o_psum", bufs=1, space="PSUM"))
            st_psum = lctx.enter_context(tc.tile_pool(name="st_psum", bufs=1, space="PSUM"))

            qk_sb = apool.tile([128, 2, 2, T], BF)   # [part, q/k, head-pair, t]
            vaug = apool.tile([128, KT, HL, 65], BF)  # token-major V + ones col
            oT = apool.tile([128, 2, T], BF)         # attn head outputs (feature-major)
            x1b = xpool.tile([128, KT, T], BF)       # pre-LN accumulator
            f1 = fpool.tile([128, KT, T], BF)        # FFN hidden (local)
            nc.vector.memset(vaug[:, :, :, 64:65], 1.0)

            def ln_stats(xsrc, tsl, tag):
                """Broadcast stats for tokens tsl of xsrc [128,KT,T] bf16.
                Returns (mu_bc, rp_bc) f32 SBUF [128,512]."""
                ps_s1 = st_psum.tile([128, 512], F32, tag="st1")
                ps_s2 = st_psum.tile([128, 512], F32, tag="st2")
                xsqs = []
                for kt in range(KT):
                    xsq = xsqp.tile([128, 512], BF, tag="xsq")
                    nc.scalar.activation(xsq, xsrc[:, kt, tsl], AF.Square)
                    xsqs.append(xsq)
                for kt in range(KT):
                    nc.tensor.matmul(ps_s1, ones128, xsrc[:, kt, tsl],
                                     start=(kt == 0), stop=(kt == KT - 1))
                for kt in range(KT):
                    nc.tensor.matmul(ps_s2, ones128, xsqs[kt],
                                     start=(kt == 0), stop=(kt == KT - 1))
                mu_bc = stp.tile([128, 512], F32, tag=f"mu{tag}")
                nc.vector.tensor_scalar(out=mu_bc, in0=ps_s1, scalar1=1.0 / D,
                                        scalar2=None, op0=ALU.mult)
                m1sq = spool.tile([128, 512], F32, tag="m1sq")
                nc.gpsimd.tensor_mul(m1sq, mu_bc, mu_bc)
                u = spool.tile([128, 512], F32, tag="uvar")
                nc.vector.scalar_tensor_tensor(
                    out=u, in0=ps_s2, scalar=1.0 / D, in1=m1sq,
                    op0=ALU.mult, op1=ALU.subtract)
                sq = spool.tile([128, 512], F32, tag="sqv")
                nc.scalar.activation(sq, u, AF.Sqrt, bias=eps128[:, 0:1])
                rp_bc = stp.tile([128, 512], F32, tag=f"rp{tag}")
                nc.vector.reciprocal(rp_bc, sq)
                return mu_bc, rp_bc

            def ln_apply(xsrc, tsl, mu_bc, rp_bc, g_sb, b_sb):
                """hb[:, :, tsl] = (xsrc - mu)*rp*g + b."""
                for kt in range(KT):
                    xm = spool.tile([128, 512], BF, tag="xm")
                    if kt % 2 == 0:
                        nc.gpsimd.tensor_sub(xm, xsrc[:, kt, tsl], mu_bc)
                    else:
                        nc.vector.tensor_sub(xm, xsrc[:, kt, tsl], mu_bc)
                    t = spool.tile([128, 512], BF, tag="lnt")
                    nc.vector.tensor_mul(t, xm, rp_bc)
                    nc.scalar.activation(
                        hb[:, kt, tsl], t, AF.Identity,
                        bias=b_sb[:, kt:kt + 1], scale=g_sb[:, kt:kt + 1])

            def phase_qkv(l, wqkv_sb, bqkv_sb, blk):
                tsl = slice(blk * 512, (blk + 1) * 512)
                for io in range(2):        # 0=q, 1=k  (feature-major out)
                    for mt in range(2):    # head pair
                        mcol = (io * 2 + mt) * 128
                        ps = mm_psum.tile([128, 512], F32, tag="mm")
                        for kt in range(KT):
                            nc.tensor.matmul(
                                ps, wqkv_sb[:, kt, mcol:mcol + 128], hb[:, kt, tsl],
                                start=(kt == 0), stop=(kt == KT - 1))
                        nc.scalar.activation(
                            qk_sb[:, io, mt, tsl], ps, AF.Identity,
                            bias=bqkv_sb[:, io * 2 + mt:io * 2 + mt + 1])
                for tm in range(4 * blk, 4 * (blk + 1)):   # v, token-major
                    ps = mm_psum.tile([128, 256], F32, tag="mm")
                    for kt in range(KT):
                        nc.tensor.matmul(
                            ps, hb[:, kt, tm * 128:(tm + 1) * 128], wqkv_sb[:, kt, 512:768],
                            start=(kt == 0), stop=(kt == KT - 1))
                    nc.scalar.activation(
                        vaug[:, tm, :, 0:64],
                        ps.rearrange("p (h e) -> p h e", h=HL), AF.Copy)

            def phase_attn(l, bqkv_sb, blk):
                t1sl = slice(blk * 512, (blk + 1) * 512)
                t2max = 4 * (blk + 1)
                for h in range(HL):
                    prow = slice(64 * (h % 2), 64 * (h % 2) + 64)
                    hm = h // 2
                    et = epool.tile([128, KT, 512], BF, tag="eT")
                    for t2t in range(t2max):
                        ps = mm_psum.tile([128, 512], F32, tag="mm")
                        nc.tensor.matmul(
                            ps,
                            qk_sb[prow, 1, hm, t2t * 128:(t2t + 1) * 128],
                            qk_sb[prow, 0, hm, t1sl],
                            start=True, stop=True)
                        nc.scalar.activation(et[:, t2t, :], ps, AF.Exp, scale=0.125)
                        j = t2t - 4 * blk
                        if j >= 0:
                            nc.vector.tensor_mul(et[:, t2t, :], et[:, t2t, :], maskq[:, j, :])
                    ps_o = o_psum.tile([65, 512], F32, tag="o")
                    for t2t in range(t2max):
                        nc.tensor.matmul(
                            ps_o, vaug[:, t2t, h, :], et[:, t2t, :],
                            start=(t2t == 0), stop=(t2t == t2max - 1))
                    rec = rpool.tile([1, 512], F32, tag="rec")
                    nc.vector.reciprocal(rec, ps_o[64:65, :])
                    ps_b = bc_psum.tile([128, 512], F32, tag="bc")
                    nc.tensor.matmul(ps_b, ones_m, rec, start=True, stop=True)
                    osb = spool.tile([64, 512], BF, tag="osb")
                    nc.scalar.copy(osb, ps_o[0:64, :])
                    psb_sb = spool.tile([64, 512], BF, tag="psb")
                    nc.vector.tensor_copy(psb_sb, ps_b[0:64, :])
                    tmp = spool.tile([64, 512], BF, tag="otmp")
                    nc.vector.tensor_mul(tmp, osb, psb_sb)
                    nc.scalar.activation(
                        oT[prow, hm, t1sl], tmp, AF.Identity,
                        bias=bqkv_sb[prow, 4 + hm:5 + hm])

            def stage_ag(blk):
                tsl = slice(blk * 512, (blk + 1) * 512)
                ag_in = dram.tile([2 * 128, 512], BF, tag="agin")
                for pt in range(2):
                    nc.sync.dma_start(ag_in[pt * 128:(pt + 1) * 128, :], oT[:, pt, tsl])
                ag_out = dram.tile([D, 512], BF, tag="agout")
                nc.gpsimd.collective_compute(
                    "AllGather", ALU.bypass, replica_groups=RG,
                    ins=[ag_in.opt()], outs=[ag_out.opt()])
                return ag_out

            def phase_oproj(l, wo_sb, bo_sb, ag_out, blk):
                """Full out-proj from gathered head outputs; x1b = oproj + bo + hb."""
                tsl = slice(blk * 512, (blk + 1) * 512)
                agh = agp.tile([128, KT, 512], BF, tag="agh")
                nc.sync.dma_start(agh, _r2(ag_out))
                for mt in range(KT):
                    ps = mm_psum.tile([128, 512], F32, tag="mm")
                    for kt in range(KT):
                        nc.tensor.matmul(
                            ps, wo_sb[:, kt, mt * 128:(mt + 1) * 128], agh[:, kt, :],
                            start=(kt == 0), stop=(kt == KT - 1))
                    nc.vector.scalar_tensor_tensor(
                        out=x1b[:, mt, tsl], in0=ps, scalar=bo_sb[:, mt:mt + 1],
                        in1=hb[:, mt, tsl], op0=ALU.add, op1=ALU.add)

            def phase_ffn(l, b1_sb, b2_sb, blk):
                tsl = slice(blk * 512, (blk + 1) * 512)
                for mt in range(KT):
                    w1_sb = wch_pool.tile([128, KT, 128], BF, tag="wch")
                    nc.sync.dma_start(w1_sb, w1r[l, mt])
                    ps = mm_psum.tile([128, 512], F32, tag="mm")
                    for kt in range(KT):
                        nc.tensor.matmul(ps, w1_sb[:, kt, :], hb[:, kt, tsl],
                                         start=(kt == 0), stop=(kt == KT - 1))
                    nc.scalar.activation(f1[:, mt, tsl], ps, AF.Relu,
                                         bias=b1_sb[:, mt:mt + 1])
                ar_in = dram.tile([D, 512], BF, tag="arin")
                for mt in range(KT):
                    w2_sb = wch_pool.tile([128, KT, 128], BF, tag="wch")
                    nc.sync.dma_start(w2_sb, w2r[l, mt])
                    ps = mm_psum.tile([128, 512], F32, tag="mm")
                    for kt in range(KT):
                        nc.tensor.matmul(ps, w2_sb[:, kt, :], f1[:, kt, tsl],
                                         start=(kt == 0), stop=(kt == KT - 1))
                    ob = spool.tile([128, 512], BF, tag="ob")
                    nc.scalar.activation(ob, ps, AF.Identity, bias=b2_sb[:, mt:mt + 1])
                    nc.sync.dma_start(ar_in[mt * 128:(mt + 1) * 128, :], ob)
                ar_out = dram.tile([D, 512], BF, tag="arout")
                nc.gpsimd.collective_compute(
                    "AllReduce", ALU.add, replica_groups=RG,
                    ins=[ar_in.opt()], outs=[ar_out.opt()])
                return ar_out

            def phase_ln2(l, ar_out, g_sb, b_sb, blk):
                """x2 = ar + hb ; LN2 -> hb (for token block blk)."""
                tsl = slice(blk * 512, (blk + 1) * 512)
                oar = oarp.tile([128, KT, 512], BF, tag="oar")
                nc.sync.dma_start(oar, _r2(ar_out))
                for kt in range(KT):
                    nc.gpsimd.tensor_add(x1b[:, kt, tsl], oar[:, kt, :], hb[:, kt, tsl])
                mu, rp = ln_stats(x1b, tsl, "b")
                ln_apply(x1b, tsl, mu, rp, g_sb, b_sb)

            # ---- layer loop, software-pipelined across the block dim ----
            pend_f1 = None   # deferred (ar_out1, g2_sb, bb2_sb) from prev layer
            for l in range(N_LAYERS):
                _h = _scope(f"L{l}.wload")
                wqkv_sb = wq_pool.tile([128, KT, 768], BF, tag="wqkv")
                nc.sync.dma_start(wqkv_sb, _r2(wqkv[l]))
                wo_sb = wo_pool.tile([128, KT, D], BF, tag="wo")
                nc.sync.dma_start(wo_sb, _r2(wo_f[l]))
                bqkv_sb = bpool.tile([128, 6], F32, tag="bqkv")
                nc.sync.dma_start(bqkv_sb, bqkv[l].rearrange("(m p) -> p m", p=128))
                bo_sb = bpool.tile([128, KT], F32, tag="bo")
                nc.sync.dma_start(bo_sb, bo[l].rearrange("(m p) -> p m", p=128))
                g1_sb = bpool.tile([128, KT], F32, tag="g1")
                nc.sync.dma_start(g1_sb, ln1g[l].rearrange("(m p) -> p m", p=128))
                bb1_sb = bpool.tile([128, KT], F32, tag="bb1")
                nc.sync.dma_start(bb1_sb, ln1b[l].rearrange("(m p) -> p m", p=128))
                b1_sb = bpool.tile([128, KT], F32, tag="b1")
                nc.sync.dma_start(b1_sb, b1[l].rearrange("(m p) -> p m", p=128))
                b2_sb = bpool.tile([128, KT], F32, tag="b2")
                nc.sync.dma_start(b2_sb, b2q[l].rearrange("(m p) -> p m", p=128))
                g2_sb = bpool.tile([128, KT], F32, tag="g2")
                nc.sync.dma_start(g2_sb, ln2g[l].rearrange("(m p) -> p m", p=128))
                bb2_sb = bpool.tile([128, KT], F32, tag="bb2")
                nc.sync.dma_start(bb2_sb, ln2b[l].rearrange("(m p) -> p m", p=128))
                _escope(_h)

                _h = _scope(f"L{l}.ab0")
                phase_qkv(l, wqkv_sb, bqkv_sb, 0)
                phase_attn(l, bqkv_sb, 0)
                ag0 = stage_ag(0)
                _escope(_h)

                if pend_f1 is not None:
                    _h = _scope(f"L{l}.f1prev")
                    phase_ln2(l - 1, *pend_f1, 1)
                    _escope(_h)
                    pend_f1 = None

                _h = _scope(f"L{l}.ab1")
                phase_qkv(l, wqkv_sb, bqkv_sb, 1)
                phase_attn(l, bqkv_sb, 1)
                ag1 = stage_ag(1)
                _escope(_h)

                _h = _scope(f"L{l}.d0")
                phase_oproj(l, wo_sb, bo_sb, ag0, 0)
                mu0, rp0 = ln_stats(x1b, slice(0, 512), "a")
                _escope(_h)
                _h = _scope(f"L{l}.d1")
                phase_oproj(l, wo_sb, bo_sb, ag1, 1)
                _escope(_h)
                _h = _scope(f"L{l}.e0")
                ln_apply(x1b, slice(0, 512), mu0, rp0, g1_sb, bb1_sb)
                ar0 = phase_ffn(l, b1_sb, b2_sb, 0)
                _escope(_h)
                _h = _scope(f"L{l}.e1")
                mu1, rp1 = ln_stats(x1b, slice(512, 1024), "a")
                ln_apply(x1b, slice(512, 1024), mu1, rp1, g1_sb, bb1_sb)
                ar1 = phase_ffn(l, b1_sb, b2_sb, 1)
                _escope(_h)
                _h = _scope(f"L{l}.f0")
                phase_ln2(l, ar0, g2_sb, bb2_sb, 0)
                _escope(_h)
                pend_f1 = (ar1, g2_sb, bb2_sb)

            _h = _scope("flast")
            phase_ln2(N_LAYERS - 1, *pend_f1, 1)
            _escope(_h)

        # ---- vocab projection + log-softmax (layer pools now closed) ----
        _h = _scope("vocab")
        if not SKIP_FINAL:
            with ExitStack() as vctx:
                lgp = vctx.enter_context(tc.tile_pool(name="lgp", bufs=2))
                vwch = vctx.enter_context(tc.tile_pool(name="vwch", bufs=4))
                vdram = vctx.enter_context(tc.tile_pool(name="vdram", bufs=2, space="DRAM"))
                va_psum = vctx.enter_context(tc.tile_pool(name="va_psum", bufs=2, space="PSUM"))

                bout_sb = const.tile([128, NVM], F32)
                nc.sync.dma_start(bout_sb, bout.rearrange("(m p) -> p m", p=128))
                for th in range(2):
                    tsl = slice(th * 512, (th + 1) * 512)
                    lg = lgp.tile([128, NVM, 512], BF, tag="lg")
                    ps_acc = va_psum.tile([1, 512], F32, tag="vacc")
                    for vm in range(NVM):
                        wv_sb = vwch.tile([128, KT, 128], BF, tag="vw")
                        nc.sync.dma_start(wv_sb, woutr[vm])
                        ps = mm_psum.tile([128, 512], F32, tag="mm")
                        for kt in range(KT):
                            nc.tensor.matmul(ps, wv_sb[:, kt, :], hb[:, kt, tsl],
                                             start=(kt == 0), stop=(kt == KT - 1))
                        nc.scalar.activation(lg[:, vm, :], ps, AF.Identity,
                                             bias=bout_sb[:, vm:vm + 1])
                        eb = spool.tile([128, 512], BF, tag="eb")
                        nc.scalar.activation(eb, lg[:, vm, :], AF.Exp)
                        nc.tensor.matmul(
                            ps_acc, ones_k, eb,
                            start=(vm == 0), stop=(vm == NVM - 1), skip_group_check=True)
                    se_row = rpool.tile([1, 512], F32, tag="serow")
                    nc.vector.tensor_copy(se_row, ps_acc)
                    se_in = vdram.tile([1, 512], F32, tag="sein")
                    nc.sync.dma_start(se_in, se_row)
                    se_out = vdram.tile([1, 512], F32, tag="seout")
                    nc.gpsimd.collective_compute(
                        "AllReduce", ALU.add, replica_groups=RG,
                        ins=[se_in.opt()], outs=[se_out.opt()])
                    se_sb = rpool.tile([1, 512], F32, tag="sesb")
                    nc.sync.dma_start(se_sb, se_out)
                    lr = rpool.tile([1, 512], F32, tag="lr")
                    nc.scalar.activation(lr, se_sb, AF.Ln)
                    psl = bc_psum.tile([128, 512], F32, tag="bc")
                    nc.tensor.matmul(psl, ones_m, lr, start=True, stop=True)
                    psl_sb = spool.tile([128, 512], F32, tag="psl")
                    nc.vector.tensor_copy(psl_sb, psl)
                    for vm in range(NVM):
                        outf = spool.tile([128, 512], F32, tag="outf")
                        if vm % 3 != 2:
                            nc.vector.tensor_sub(outf, lg[:, vm, :], psl_sb)
                        else:
                            nc.gpsimd.tensor_sub(outf, lg[:, vm, :], psl_sb)
                        nc.sync.dma_start(out[vm * 128:(vm + 1) * 128, tsl], outf)
        else:
            # debug: dump hb as f32 into the first D rows of out
            for kt in range(KT):
                dbg = spool.tile([128, T], F32, tag="outf")
                nc.scalar.activation(dbg, hb[:, kt, :], AF.Copy)
                nc.sync.dma_start(out[kt * 128:(kt + 1) * 128, :], dbg)
        _escope(_h)

    nsplit = split_sync_waits(nc)
    print(f"split_sync_waits: {nsplit} NOPs inserted")
    return nc


def _bf16(a):
    return np.asarray(a, dtype=ml_dtypes.bfloat16)


def make_in_maps(x, tok_emb, pos_emb, wq, bq, wk, bk, wv, bv, wo, bo,
                 ln1_g, ln1_b, w1, b1, w2, b2, ln2_g, ln2_b, w_out, b_out):
    """Shard full inputs -> per-core input maps (host-reshaped weights)."""
    LE = wq.shape[0]
    wo_full = _bf16(wo)
    per_r = []
    for r in range(TP):
        hs = slice(HL * r, HL * (r + 1))
        wqkv_r = np.concatenate(
            [
                wq[:, hs].transpose(0, 2, 1, 3).reshape(LE, D, HL * DH),
                wk[:, hs].transpose(0, 2, 1, 3).reshape(LE, D, HL * DH),
                wv[:, hs].transpose(0, 2, 1, 3).reshape(LE, D, HL * DH),
            ],
            axis=2,
        )
        bqkv_r = np.concatenate(
            [bq[:, hs].reshape(LE, -1), bk[:, hs].reshape(LE, -1),
             bv[:, hs].reshape(LE, -1)], axis=1,
        )
        fs = slice(FFL * r, FFL * (r + 1))
        vs = slice(VL * r, VL * (r + 1))
        w1s = np.ascontiguousarray(w1[:, :, fs]).reshape(LE, KT, 128, KT, 128)
        w1rr = np.ascontiguousarray(np.transpose(w1s, (0, 3, 2, 1, 4)))
        w2s = np.ascontiguousarray(w2[:, fs, :]).reshape(LE, KT, 128, KT, 128)
        w2rr = np.ascontiguousarray(np.transpose(w2s, (0, 3, 2, 1, 4)))
        wout_r = np.zeros((D, VLP), np.float32)
        wout_r[:, :VL] = w_out[:, vs]
        woutrr = np.ascontiguousarray(
            np.transpose(wout_r.reshape(KT, 128, NVM, 128), (2, 1, 0, 3)))
        bout_r = np.full((VLP,), -1e30, np.float32)
        bout_r[:VL] = b_out[vs]
        per_r.append(dict(
            wqkv=_bf16(wqkv_r),
            bqkv=np.ascontiguousarray(bqkv_r, np.float32),
            wo_f=wo_full,
            bo=np.ascontiguousarray(bo, np.float32),
            ln1g=np.ascontiguousarray(ln1_g, np.float32),
            ln1b=np.ascontiguousarray(ln1_b, np.float32),
            w1r=_bf16(w1rr),
            b1=np.ascontiguousarray(b1[:, fs], np.float32),
            w2r=_bf16(w2rr),
            b2q=np.ascontiguousarray(b2 / TP, np.float32),
            ln2g=np.ascontiguousarray(ln2_g, np.float32),
            ln2b=np.ascontiguousarray(ln2_b, np.float32),
            woutr=_bf16(woutrr),
            bout=bout_r,
        ))
    in_maps = []
    for c in range(8):
        g, r = c // TP, c % TP
        emb = tok_emb[x[g]] + pos_emb[:S]          # [S, D]
        m = dict(per_r[r])
        m["h0b"] = _bf16(np.ascontiguousarray(emb.T))
        in_maps.append(m)
    return in_maps


_CACHED = {}


def kernel(**inputs):
    inputs = {k: np.asarray(v) for k, v in inputs.items()}
    if "nc" not in _CACHED:
        _CACHED["nc"] = build_program()
    nc = _CACHED["nc"]
    in_maps = make_in_maps(**inputs)
    trace = os.environ.get("BASS_GPT_TRACE", "0") == "1"
    res = run_bass_kernel_spmd(
        nc, in_maps, core_ids=list(range(8)), trace=trace,
    )
    if trace:
        print(f"HW exec time: {res.exec_time_ns} ns")
        _CACHED["last_result"] = res
    results = res.results
    full = np.empty((B, S, V), np.float32)
    for c in range(8):
        g, r = c // TP, c % TP
        full[g, :, VL * r: VL * (r + 1)] = results[c]["out"][:VL, :].T
    return full



# revision 17
# speedup vs baseline: 1.3467x; 1.3467x over previous
"""GPT-style transformer forward on 8 Trainium2 NeuronCores.

Sharding: data-parallel over batch (2 groups of 4 cores), tensor-parallel
within each group (heads / FFN hidden / vocab columns split 4 ways).
Device activations are feature-major [feature, token] so all matmuls run
without transposes.

v3: fp8e4 DoubleRow matmuls (2x PE throughput) for QKV/out-proj/FFN/vocab
with host-prescaled weights (x64) compensated in the drains; exp(-ln(x))
reciprocals on the Activation engine (exact DVE reciprocal is 4.3us);
FFN residual add folded into the AllReduce inputs (hb/4 + b2/4); fp8
AllGather of attention head outputs; bf16 output tensor; elementwise work
rebalanced across DVE/Pool/Activation; full-layer weight prefetch via
double-buffered pools.
"""

import os
from contextlib import ExitStack

import numpy as np
import ml_dtypes

import concourse.bass as bass
import concourse.mybir as mybir
import concourse.tile as tile
from concourse.bass_utils import run_bass_kernel_spmd
from concourse.vector_clock import ScopedClock


def _drain_and_barrier(self, tick_clock, wait_clock):
    """The walrus build here encodes Drain/NoOp as TPB_CTRL with at most one
    sync-wait slot; Tile's stock tail attaches all outstanding waits to the
    Drain and fails codegen. Split the waits one-per-NOP instead."""
    nop_inst = self.nc.sync.nop(nofuse=True)
    wait_clock.add_sem_waits(nop_inst.ins, ScopedClock({None: tick_clock.global_clock}))
    si = nop_inst.ins.sync_info
    if si is not None and len(si.on_wait) > 1:
        waits = list(si.on_wait)
        nop_inst.ins.sync_info = mybir.SyncInfo(on_wait=waits[:1], on_update=list(si.on_update))
        for w in waits[1:]:
            n2 = self.nc.sync.nop(nofuse=True)
            n2.ins.sync_info = mybir.SyncInfo(on_wait=[w], on_update=[])
    self.nc.sync.drain()
    self.nc.all_engine_barrier()
    assert self.sems is not None
    popped = self.nc._tile_sem_poison_stack.pop()
    assert popped is self._sem_poison
    self.nc.clear_and_free_semaphores(list(self.sems.allocated().values()))
    self.nc.all_engine_barrier()


tile.TileContext._drain_and_barrier = _drain_and_barrier

_MAX_WAITS = 1  # this walrus build caps sync-waits per instruction


def split_sync_waits(nc):
    """Hoist excess on_wait entries onto same-engine NOPs inserted before the
    instruction (engine queues execute in program order, so semantics hold)."""
    n = 0
    for bb in nc.main_func.blocks:
        insts = bb.instructions
        new_list = []
        for inst in insts:
            si = getattr(inst, "sync_info", None)
            if si is not None and len(si.on_wait) > _MAX_WAITS:
                waits = list(si.on_wait)
                for w in waits[:-_MAX_WAITS]:
                    n += 1
                    new_list.append(mybir.InstNoOp(
                        name=f"{inst.name}-sw{n}",
                        sync_info=mybir.SyncInfo(on_wait=[w], on_update=[]),
                        bass_nofuse=True,
                        engine=inst.engine,
                    ))
                inst.sync_info = mybir.SyncInfo(
                    on_wait=waits[-_MAX_WAITS:], on_update=list(si.on_update)
                )
            new_list.append(inst)
        if len(new_list) != len(insts):
            bb.instructions[:] = new_list
    return n


# Model dims (hardcoded per problem spec)
L_FULL, H, D, V, SMAX = 8, 16, 1024, 32000, 1024
DH = D // H          # 64
FF = 4 * D           # 4096
B, S = 2, 1024
T = S                # tokens per group (one batch element per group)
TP = 4               # tensor-parallel degree within a group
HL = H // TP         # 4 local heads
FFL = FF // TP       # 1024 local FFN cols
VL = V // TP         # 8000 local vocab cols
VLP = 8064           # padded to 63*128
NVM = VLP // 128     # 63 vocab m-tiles
EPS = 1e-5
KT = D // 128        # 8 k-tiles over model dim
KT2 = KT // 2        # 4 fp8 DoubleRow k-pairs
NB = T // 512        # 2 token blocks of 512

WS = 64.0            # fp8 weight prescale
WSI = 1.0 / WS

BF = mybir.dt.bfloat16
F32 = mybir.dt.float32
F8 = mybir.dt.float8e4
AF = mybir.ActivationFunctionType
ALU = mybir.AluOpType
DR = mybir.MatmulPerfMode.DoubleRow

RG = [[0, 1, 2, 3], [4, 5, 6, 7]]

N_LAYERS = int(os.environ.get("BASS_GPT_LAYERS", str(L_FULL)))
SKIP_FINAL = os.environ.get("BASS_GPT_SKIP_FINAL", "0") == "1"

# Set from the actual inputs before build: when LN gains are all-ones /
# biases all-zero (true for this model family), the g/b passes and bias
# adds are dropped and scale-invariance folds the fp8 compensation into
# the residual adds.
FOLD_LN = True
FOLD_BIAS = True


def _r2(ap):
    """[ (kt p) n ] -> [p kt n] view of a DRAM 2-D tensor (p=128)."""
    return ap.rearrange("(kt p) n -> p kt n", p=128)


def _bc(ap, dim, n):
    """Insert a stride-0 broadcast dim of size n at position `dim`."""
    newap = [list(d) for d in ap.ap]
    newap.insert(dim, [0, n])
    return bass.AP(ap.tensor, ap.offset, newap)


def build_program():
    nc = bass.Bass("TRN2")

    # ---- DRAM parameters (per-core shards, host-reshaped) ----
    h0b = nc.declare_dram_parameter("h0b", [D, T], BF, isOutput=False)
    h0b8 = nc.declare_dram_parameter("h0b8", [128, KT, T], F8, isOutput=False)
    wqkv8 = nc.declare_dram_parameter("wqkv8", [N_LAYERS, 128, KT2, 2, 3 * HL * DH], F8, isOutput=False)
    bqkv = nc.declare_dram_parameter("bqkv", [N_LAYERS, 3 * HL * DH], F32, isOutput=False)
    wo8 = nc.declare_dram_parameter("wo8", [N_LAYERS, 128, KT2, 2, D], F8, isOutput=False)
    bo = nc.declare_dram_parameter("bo", [N_LAYERS, D], F32, isOutput=False)
    ln1g = nc.declare_dram_parameter("ln1g", [N_LAYERS, D], F32, isOutput=False)
    ln1b = nc.declare_dram_parameter("ln1b", [N_LAYERS, D], F32, isOutput=False)
    w1r = nc.declare_dram_parameter("w1r", [N_LAYERS, 128, KT, FFL], BF, isOutput=False)
    b1 = nc.declare_dram_parameter("b1", [N_LAYERS, FFL], F32, isOutput=False)
    w2r = nc.declare_dram_parameter("w2r", [N_LAYERS, 128, KT, D], BF, isOutput=False)
    b2q4 = nc.declare_dram_parameter("b2q4", [N_LAYERS, D], F32, isOutput=False)
    ln2g = nc.declare_dram_parameter("ln2g", [N_LAYERS, D], F32, isOutput=False)
    ln2b = nc.declare_dram_parameter("ln2b", [N_LAYERS, D], F32, isOutput=False)
    woutr = nc.declare_dram_parameter("woutr", [NVM, 128, KT, 128], BF, isOutput=False)
    bout = nc.declare_dram_parameter("bout", [VLP], F32, isOutput=False)
    out = nc.declare_dram_parameter("out", [VLP, T], BF, isOutput=True)

    with ExitStack() as ctx:
        tc = ctx.enter_context(tile.TileContext(nc))

        def _scope(name):
            sid, _ = nc.enter_named_scope(name, False)
            return (name, sid)

        def _escope(h):
            nc.leave_named_scope(h[0], h[1], False)

        # ---- outer pools (live whole program) ----
        const = ctx.enter_context(tc.tile_pool(name="const", bufs=1))
        hpool = ctx.enter_context(tc.tile_pool(name="hpool", bufs=1))
        spool = ctx.enter_context(tc.tile_pool(name="spool", bufs=2))
        rpool = ctx.enter_context(tc.tile_pool(name="rpool", bufs=2))
        mm_psum = ctx.enter_context(tc.tile_pool(name="mm_psum", bufs=3, space="PSUM"))
        bc_psum = ctx.enter_context(tc.tile_pool(name="bc_psum", bufs=1, space="PSUM"))
        dram = ctx.enter_context(tc.tile_pool(name="dram", bufs=2, space="DRAM"))

        # ---- constants ----
        ones_k = const.tile([128, 1], BF)       # lhsT for partition-sum (M=1)
        nc.vector.memset(ones_k, 1.0)
        ones128 = const.tile([128, 128], BF)    # lhsT for bcast partition-sum (M=128)
        nc.vector.memset(ones128, 1.0)
        ones_m = const.tile([1, 128], BF)       # lhsT for broadcast (K=1, M=128)
        nc.vector.memset(ones_m, 1.0)
        eps128 = const.tile([128, 1], F32)
        nc.vector.memset(eps128, float(EPS))
        # causal keep-masks: variant j keeps where t1f - t2p - 128*j >= 0
        maskq = const.tile([128, 4, 512], BF)
        nc.gpsimd.memset(maskq, 1.0)
        for j in range(4):
            nc.gpsimd.affine_select(
                out=maskq[:, j, :], in_=maskq[:, j, :],
                compare_op=ALU.is_ge, fill=0.0,
                base=-128 * j, pattern=[[1, 512]], channel_multiplier=-1,
            )

        # ---- persistent activation state ----
        hb = hpool.tile([128, KT, T], BF)       # residual stream (feature-major)
        nc.sync.dma_start(hb, _r2(h0b))
        hb8 = hpool.tile([128, KT, T], F8)      # fp8 copy for DR matmul rhs
        nc.sync.dma_start(hb8, h0b8[:])

        with ExitStack() as lctx:
            apool = lctx.enter_context(tc.tile_pool(name="apool", bufs=1))
            xpool = lctx.enter_context(tc.tile_pool(name="xpool", bufs=1))
            fpool = lctx.enter_context(tc.tile_pool(name="fpool", bufs=1))
            epool = lctx.enter_context(tc.tile_pool(name="epool", bufs=2))
            wq_pool = lctx.enter_context(tc.tile_pool(name="wq_pool", bufs=2))
            wo_pool = lctx.enter_context(tc.tile_pool(name="wo_pool", bufs=2))
            w1_pool = lctx.enter_context(tc.tile_pool(name="w1_pool", bufs=1))
            w2_pool = lctx.enter_context(tc.tile_pool(name="w2_pool", bufs=1))
            agp = lctx.enter_context(tc.tile_pool(name="agp", bufs=2))
            oarp = lctx.enter_context(tc.tile_pool(name="oarp", bufs=1))
            xsqb = lctx.enter_context(tc.tile_pool(name="xsqb", bufs=1))
            bpool = lctx.enter_context(tc.tile_pool(name="bpool", bufs=2))
            stp = lctx.enter_context(tc.tile_pool(name="stp", bufs=1))
            o_psum = lctx.enter_context(tc.tile_pool(name="o_psum", bufs=2, space="PSUM"))
            st_psum = lctx.enter_context(tc.tile_pool(name="st_psum", bufs=1, space="PSUM"))

            qk_sb = apool.tile([128, 2, 2, T], BF)   # [part, q/k, head-pair, t]
            vaug = apool.tile([128, KT, HL, 65], BF)  # token-major V + ones col
            oT8 = apool.tile([128, 2, T], F8)        # attn head outputs (feature-major)
            x1b = xpool.tile([128, KT, T], BF)       # pre-LN1 accumulator
            nc.vector.memset(vaug[:, :, :, 64:65], 1.0)

            def ln_stats(xsrc, tsl, tag):
                """Broadcast stats for tokens tsl of xsrc [128,KT,*] bf16.
                Returns (mu_bc, rp_bc) SBUF [128,512]; rp via exp(-.5 ln).
                Scale-invariant consumers let xsrc carry any uniform scale."""
                ps_s1 = st_psum.tile([128, 512], F32, tag="st1")
                ps_s2 = st_psum.tile([128, 512], F32, tag="st2")
                xsq = xsqb.tile([128, KT, 512], BF, tag="xsq")
                nc.vector.tensor_mul(xsq[:, 0:5, :], xsrc[:, 0:5, tsl], xsrc[:, 0:5, tsl])
                nc.gpsimd.tensor_mul(xsq[:, 5:8, :], xsrc[:, 5:8, tsl], xsrc[:, 5:8, tsl])
                for kt in range(KT):
                    nc.tensor.matmul(ps_s1, ones128, xsrc[:, kt, tsl],
                                     start=(kt == 0), stop=(kt == KT - 1))
                for kt in range(KT):
                    nc.tensor.matmul(ps_s2, ones128, xsq[:, kt, :],
                                     start=(kt == 0), stop=(kt == KT - 1))
                mu_bc = stp.tile([128, 512], BF, tag=f"mu{tag}")
                nc.vector.tensor_scalar(out=mu_bc, in0=ps_s1, scalar1=1.0 / D,
                                        scalar2=None, op0=ALU.mult)
                m1sq = spool.tile([128, 512], BF, tag="m1sq")
                nc.gpsimd.tensor_mul(m1sq, mu_bc, mu_bc)
                u = spool.tile([128, 512], BF, tag="uvar")
                nc.vector.scalar_tensor_tensor(
                    out=u, in0=ps_s2, scalar=1.0 / D, in1=m1sq,
                    op0=ALU.mult, op1=ALU.subtract)
                lnu = spool.tile([128, 512], BF, tag="lnu")
                nc.scalar.activation(lnu, u, AF.Ln, bias=eps128[:, 0:1])
                rp_bc = stp.tile([128, 512], BF, tag=f"rp{tag}")
                nc.scalar.activation(rp_bc, lnu, AF.Exp, scale=-0.5)
                return mu_bc, rp_bc

            def ln_apply(xsrc, tsl, mu_bc, rp_bc, g_sb, b_sb, hb_tsl=None):
                """hb[:, :, hb_tsl] = (xsrc - mu)*rp*g + b ; hb8 = fp8(hb).
                Batched over kt with stride-0 broadcast of mu/rp; when
                FOLD_LN (g==1, b==0) the g/b passes are dropped."""
                if hb_tsl is None:
                    hb_tsl = tsl
                xm = xsqb.tile([128, KT, 512], BF, tag="xmb")
                nc.vector.tensor_sub(xm[:, 0:5, :], xsrc[:, 0:5, tsl], _bc(mu_bc, 1, 5))
                nc.gpsimd.tensor_sub(xm[:, 5:8, :], xsrc[:, 5:8, tsl], _bc(mu_bc, 1, 3))
                if FOLD_LN:
                    nc.vector.tensor_mul(hb[:, 0:5, hb_tsl], xm[:, 0:5, :], _bc(rp_bc, 1, 5))
                    nc.gpsimd.tensor_mul(hb[:, 5:8, hb_tsl], xm[:, 5:8, :], _bc(rp_bc, 1, 3))
                else:
                    nc.vector.tensor_mul(xm[:, 0:5, :], xm[:, 0:5, :], _bc(rp_bc, 1, 5))
                    nc.gpsimd.tensor_mul(xm[:, 5:8, :], xm[:, 5:8, :], _bc(rp_bc, 1, 3))
                    g3 = _bc(g_sb[:, 0:KT], 2, 512)
                    b3 = _bc(b_sb[:, 0:KT], 2, 512)
                    nc.vector.tensor_mul(xm, xm, g3)
                    nc.vector.tensor_add(hb[:, :, hb_tsl], xm, b3)
                nc.vector.tensor_copy(hb8[:, 0:5, hb_tsl], hb[:, 0:5, hb_tsl])
                nc.vector.tensor_copy(hb8[:, 5:8, hb_tsl], hb[:, 5:8, hb_tsl])

            def phase_qkv(l, wqkv_sb, bqkv_sb, blk):
                tsl = slice(blk * 512, (blk + 1) * 512)
                for io in range(2):        # 0=q, 1=k  (feature-major out)
                    for mt in range(2):    # head pair
                        mcol = (io * 2 + mt) * 128
                        ps = mm_psum.tile([128, 512], F32, tag="mm")
                        for j in range(KT2):
                            nc.tensor.matmul(
                                ps, wqkv_sb[:, j, :, mcol:mcol + 128],
                                hb8[:, 2 * j:2 * j + 2, tsl],
                                start=(j == 0), stop=(j == KT2 - 1), perf_mode=DR)
                        nc.vector.tensor_scalar(
                            out=qk_sb[:, io, mt, tsl], in0=ps,
                            scalar1=WSI, scalar2=bqkv_sb[:, io * 2 + mt:io * 2 + mt + 1],
                            op0=ALU.mult, op1=ALU.add)
                for tm in range(4 * blk, 4 * (blk + 1)):   # v, token-major
                    ps = mm_psum.tile([128, 256], F32, tag="mm")
                    for j in range(KT2):
                        nc.tensor.matmul(
                            ps, hb8[:, 2 * j:2 * j + 2, tm * 128:(tm + 1) * 128],
                            wqkv_sb[:, j, :, 512:768],
                            start=(j == 0), stop=(j == KT2 - 1), perf_mode=DR)
                    nc.vector.tensor_scalar(
                        out=vaug[:, tm, :, 0:64],
                        in0=ps.rearrange("p (h e) -> p h e", h=HL),
                        scalar1=WSI, scalar2=None, op0=ALU.mult)

            def phase_attn(l, bqkv_sb, blk):
                t1sl = slice(blk * 512, (blk + 1) * 512)
                t2max = 4 * (blk + 1)
                for h in range(HL):
                    prow = slice(64 * (h % 2), 64 * (h % 2) + 64)
                    hm = h // 2
                    et = epool.tile([128, KT, 512], BF, tag="eT")
                    for t2t in range(t2max):
                        ps = mm_psum.tile([128, 512], F32, tag="mm")
                        nc.tensor.matmul(
                            ps,
                            qk_sb[prow, 1, hm, t2t * 128:(t2t + 1) * 128],
                            qk_sb[prow, 0, hm, t1sl],
                            start=True, stop=True)
                        nc.scalar.activation(et[:, t2t, :], ps, AF.Exp, scale=0.125)
                    dg = slice(4 * blk, 4 * blk + 4)
                    if h % 2 == 0:
                        nc.vector.tensor_mul(et[:, dg, :], et[:, dg, :], maskq)
                    else:
                        nc.gpsimd.tensor_mul(et[:, dg, :], et[:, dg, :], maskq)
                    ps_o = o_psum.tile([65, 512], F32, tag="o")
                    for t2t in range(t2max):
                        nc.tensor.matmul(
                            ps_o, vaug[:, t2t, h, :], et[:, t2t, :],
                            start=(t2t == 0), stop=(t2t == t2max - 1))
                    # 1/Z via exp(-ln(Z)) on the Activation engine
                    lnz = rpool.tile([1, 512], F32, tag="lnz")
                    nc.scalar.activation(lnz, ps_o[64:65, :], AF.Ln)
                    rec = rpool.tile([1, 512], BF, tag="rec")
                    nc.scalar.activation(rec, lnz, AF.Exp, scale=-1.0)
                    ps_b = bc_psum.tile([64, 512], F32, tag="bc")
                    nc.tensor.matmul(ps_b, ones_m[:, 0:64], rec, start=True, stop=True)
                    psb_sb = spool.tile([64, 512], BF, tag="psb")
                    nc.vector.tensor_copy(psb_sb, ps_b)
                    tmp = spool.tile([64, 512], BF, tag="otmp")
                    nc.vector.tensor_mul(tmp, ps_o[0:64, :], psb_sb)
                    nc.vector.tensor_scalar(
                        out=oT8[prow, hm, t1sl], in0=tmp,
                        scalar1=bqkv_sb[prow, 4 + hm:5 + hm], scalar2=None,
                        op0=ALU.add)

            def stage_ag(blk):
                tsl = slice(blk * 512, (blk + 1) * 512)
                ag_in = dram.tile([2 * 128, 512], F8, tag="agin")
                for pt in range(2):
                    nc.sync.dma_start(ag_in[pt * 128:(pt + 1) * 128, :], oT8[:, pt, tsl])
                ag_out = dram.tile([D, 512], F8, tag="agout")
                nc.gpsimd.collective_compute(
                    "AllGather", ALU.bypass, replica_groups=RG,
                    ins=[ag_in.opt()], outs=[ag_out.opt()])
                return ag_out

            def phase_oproj(l, wo_sb, bo_sb, ag_out, blk):
                """Full out-proj from gathered head outputs; x1b = oproj*WSI + bo + hb."""
                tsl = slice(blk * 512, (blk + 1) * 512)
                agh = agp.tile([128, KT, 512], F8, tag="agh")
                nc.sync.dma_start(agh, _r2(ag_out))
                for mt in range(KT):
                    ps = mm_psum.tile([128, 512], F32, tag="mm")
                    for j in range(KT2):
                        nc.tensor.matmul(
                            ps, wo_sb[:, j, :, mt * 128:(mt + 1) * 128],
                            agh[:, 2 * j:2 * j + 2, :],
                            start=(j == 0), stop=(j == KT2 - 1), perf_mode=DR)
                    if FOLD_BIAS:
                        # x1b = WS*(oproj + hb); LN1 is scale-invariant
                        nc.vector.scalar_tensor_tensor(
                            out=x1b[:, mt, tsl], in0=hb[:, mt, tsl], scalar=WS,
                            in1=ps, op0=ALU.mult, op1=ALU.add)
                    else:
                        op = spool.tile([128, 512], BF, tag="ob")
                        nc.vector.tensor_scalar(
                            out=op, in0=ps, scalar1=WSI, scalar2=bo_sb[:, mt:mt + 1],
                            op0=ALU.mult, op1=ALU.add)
                        if mt % 2 == 0:
                            nc.gpsimd.tensor_add(x1b[:, mt, tsl], op, hb[:, mt, tsl])
                        else:
                            nc.vector.tensor_add(x1b[:, mt, tsl], op, hb[:, mt, tsl])

            def phase_ffn(l, w1_sb, w2_sb, b1_sb, b2q4_sb, blk):
                tsl = slice(blk * 512, (blk + 1) * 512)
                f1 = fpool.tile([128, KT, 512], BF, tag="f1")
                for mt in range(KT):
                    ps = mm_psum.tile([128, 512], F32, tag="mm")
                    for kt in range(KT):
                        nc.tensor.matmul(
                            ps, w1_sb[:, kt, mt * 128:(mt + 1) * 128],
                            hb[:, kt, tsl],
                            start=(kt == 0), stop=(kt == KT - 1))
                    nc.scalar.activation(f1[:, mt, :], ps, AF.Relu,
                                         bias=b1_sb[:, mt:mt + 1])
                # split AllReduce into kt halves so LN2 stats start early
                ar_outs = []
                for half in range(2):
                    ar_in = dram.tile([D // 2, 512], BF, tag=f"arin{half}")
                    for mi in range(4):
                        mt = half * 4 + mi
                        ps = mm_psum.tile([128, 512], F32, tag="mm")
                        for kt in range(KT):
                            nc.tensor.matmul(
                                ps, w2_sb[:, kt, mt * 128:(mt + 1) * 128],
                                f1[:, kt, :],
                                start=(kt == 0), stop=(kt == KT - 1))
                        if FOLD_BIAS:
                            # ob = hb/4 + ff_partial -> AR yields x2 exactly
                            ob = spool.tile([128, 512], BF, tag="ob")
                            nc.vector.scalar_tensor_tensor(
                                out=ob, in0=hb[:, mt, tsl], scalar=0.25,
                                in1=ps, op0=ALU.mult, op1=ALU.add)
                        else:
                            hq = spool.tile([128, 512], BF, tag="xm")
                            nc.vector.tensor_scalar(
                                out=hq, in0=hb[:, mt, tsl], scalar1=0.25,
                                scalar2=b2q4_sb[:, mt:mt + 1], op0=ALU.mult, op1=ALU.add)
                            ob = spool.tile([128, 512], BF, tag="ob")
                            nc.vector.scalar_tensor_tensor(
                                out=ob, in0=ps, scalar=WSI, in1=hq,
                                op0=ALU.mult, op1=ALU.add)
                        nc.sync.dma_start(ar_in[mi * 128:(mi + 1) * 128, :], ob)
                    ar_out = dram.tile([D // 2, 512], BF, tag=f"arout{half}")
                    nc.gpsimd.collective_compute(
                        "AllReduce", ALU.add, replica_groups=RG,
                        ins=[ar_in.opt()], outs=[ar_out.opt()])
                    ar_outs.append(ar_out)
                return ar_outs

            def phase_ln2(l, ar_outs, g_sb, b_sb, blk):
                """AR output already includes the (scaled) residual; LN2 -> hb."""
                tsl = slice(blk * 512, (blk + 1) * 512)
                oar = oarp.tile([128, KT, 512], BF, tag="oar")
                nc.sync.dma_start(oar[:, 0:4, :], _r2(ar_outs[0]))
                nc.sync.dma_start(oar[:, 4:8, :], _r2(ar_outs[1]))
                mu, rp = ln_stats(oar, slice(0, 512), "b")
                ln_apply(oar, slice(0, 512), mu, rp, g_sb, b_sb, hb_tsl=tsl)

            # ---- layer loop, software-pipelined across the block dim ----
            pend_f1 = None   # deferred (ar_out1, g2_sb, bb2_sb) from prev layer
            for l in range(N_LAYERS):
                _h = _scope(f"L{l}.wload")
                wqkv_sb = wq_pool.tile([128, KT2, 2, 768], F8, tag="wqkv")
                nc.sync.dma_start(wqkv_sb, wqkv8[l])
                wo_sb = wo_pool.tile([128, KT2, 2, D], F8, tag="wo")
                nc.sync.dma_start(wo_sb, wo8[l])
                w1_sb = w1_pool.tile([128, KT, FFL], BF, tag="w1")
                nc.sync.dma_start(w1_sb, w1r[l])
                w2_sb = w2_pool.tile([128, KT, D], BF, tag="w2")
                nc.sync.dma_start(w2_sb, w2r[l])
                bqkv_sb = bpool.tile([128, 6], F32, tag="bqkv")
                nc.sync.dma_start(bqkv_sb, bqkv[l].rearrange("(m p) -> p m", p=128))
                bo_sb = bpool.tile([128, KT], F32, tag="bo")
                nc.sync.dma_start(bo_sb, bo[l].rearrange("(m p) -> p m", p=128))
                g1_sb = bpool.tile([128, KT], F32, tag="g1")
                nc.sync.dma_start(g1_sb, ln1g[l].rearrange("(m p) -> p m", p=128))
                bb1_sb = bpool.tile([128, KT], F32, tag="bb1")
                nc.sync.dma_start(bb1_sb, ln1b[l].rearrange("(m p) -> p m", p=128))
                b1_sb = bpool.tile([128, KT], F32, tag="b1")
                nc.sync.dma_start(b1_sb, b1[l].rearrange("(m p) -> p m", p=128))
                b2q4_sb = bpool.tile([128, KT], F32, tag="b2")
                nc.sync.dma_start(b2q4_sb, b2q4[l].rearrange("(m p) -> p m", p=128))
                g2_sb = bpool.tile([128, KT], F32, tag="g2")
                nc.sync.dma_start(g2_sb, ln2g[l].rearrange("(m p) -> p m", p=128))
                bb2_sb = bpool.tile([128, KT], F32, tag="bb2")
                nc.sync.dma_start(bb2_sb, ln2b[l].rearrange("(m p) -> p m", p=128))
                _escope(_h)

                _h = _scope(f"L{l}.ab0")
                phase_qkv(l, wqkv_sb, bqkv_sb, 0)
                phase_attn(l, bqkv_sb, 0)
                ag0 = stage_ag(0)
                _escope(_h)

                if pend_f1 is not None:
                    _h = _scope(f"L{l}.f1prev")
                    phase_ln2(l - 1, *pend_f1, 1)
                    _escope(_h)
                    pend_f1 = None

                _h = _scope(f"L{l}.ab1")
                phase_qkv(l, wqkv_sb, bqkv_sb, 1)
                phase_attn(l, bqkv_sb, 1)
                ag1 = stage_ag(1)
                _escope(_h)

                _h = _scope(f"L{l}.d0")
                phase_oproj(l, wo_sb, bo_sb, ag0, 0)
                mu0, rp0 = ln_stats(x1b, slice(0, 512), "a")
                _escope(_h)
                _h = _scope(f"L{l}.d1")
                phase_oproj(l, wo_sb, bo_sb, ag1, 1)
                _escope(_h)
                _h = _scope(f"L{l}.e0")
                ln_apply(x1b, slice(0, 512), mu0, rp0, g1_sb, bb1_sb)
                ar0 = phase_ffn(l, w1_sb, w2_sb, b1_sb, b2q4_sb, 0)
                _escope(_h)
                _h = _scope(f"L{l}.e1")
                mu1, rp1 = ln_stats(x1b, slice(512, 1024), "a")
                ln_apply(x1b, slice(512, 1024), mu1, rp1, g1_sb, bb1_sb)
                ar1 = phase_ffn(l, w1_sb, w2_sb, b1_sb, b2q4_sb, 1)
                _escope(_h)
                _h = _scope(f"L{l}.f0")
                phase_ln2(l, ar0, g2_sb, bb2_sb, 0)
                _escope(_h)
                pend_f1 = (ar1, g2_sb, bb2_sb)

            _h = _scope("flast")
            phase_ln2(N_LAYERS - 1, *pend_f1, 1)
            _escope(_h)

        # ---- vocab projection + log-softmax (layer pools now closed) ----
        _h = _scope("vocab")
        if not SKIP_FINAL:
            with ExitStack() as vctx:
                lgp = vctx.enter_context(tc.tile_pool(name="lgp", bufs=2))
                vwch = vctx.enter_context(tc.tile_pool(name="vwch", bufs=6))
                vspool = vctx.enter_context(tc.tile_pool(name="vspool", bufs=2))
                vdram = vctx.enter_context(tc.tile_pool(name="vdram", bufs=2, space="DRAM"))
                va_psum = vctx.enter_context(tc.tile_pool(name="va_psum", bufs=2, space="PSUM"))

                bout_sb = const.tile([128, NVM], F32)
                nc.sync.dma_start(bout_sb, bout.rearrange("(m p) -> p m", p=128))
                for th in range(2):
                    tsl = slice(th * 512, (th + 1) * 512)
                    lg = lgp.tile([128, NVM, 512], BF, tag="lg")
                    ps_acc = va_psum.tile([1, 512], F32, tag="vacc")
                    for vm in range(NVM):
                        wv_sb = vwch.tile([128, KT, 128], BF, tag="vw")
                        nc.sync.dma_start(wv_sb, woutr[vm])
                        ps = mm_psum.tile([128, 512], F32, tag="mm")
                        for kt in range(KT):
                            nc.tensor.matmul(ps, wv_sb[:, kt, :], hb[:, kt, tsl],
                                             start=(kt == 0), stop=(kt == KT - 1))
                        if vm % 2 == 0:
                            nc.vector.tensor_scalar(
                                out=lg[:, vm, :], in0=ps, scalar1=bout_sb[:, vm:vm + 1],
                                scalar2=None, op0=ALU.add)
                        else:
                            nc.scalar.activation(lg[:, vm, :], ps, AF.Identity,
                                                 bias=bout_sb[:, vm:vm + 1])
                        eb = vspool.tile([128, 512], BF, tag="eb")
                        nc.scalar.activation(eb, ps, AF.Exp,
                                             bias=bout_sb[:, vm:vm + 1])
                        nc.tensor.matmul(
                            ps_acc, ones_k, eb,
                            start=(vm == 0), stop=(vm == NVM - 1), skip_group_check=True)
                    se_row = rpool.tile([1, 512], F32, tag="serow")
                    nc.vector.tensor_copy(se_row, ps_acc)
                    se_in = vdram.tile([1, 512], F32, tag="sein")
                    nc.sync.dma_start(se_in, se_row)
                    se_out = vdram.tile([1, 512], F32, tag="seout")
                    nc.gpsimd.collective_compute(
                        "AllReduce", ALU.add, replica_groups=RG,
                        ins=[se_in.opt()], outs=[se_out.opt()])
                    se_sb = rpool.tile([1, 512], F32, tag="sesb")
                    nc.sync.dma_start(se_sb, se_out)
                    lr = rpool.tile([1, 512], BF, tag="lr")
                    nc.scalar.activation(lr, se_sb, AF.Ln)
                    psl = bc_psum.tile([128, 512], F32, tag="bc")
                    nc.tensor.matmul(psl, ones_m, lr, start=True, stop=True)
                    psl_sb = vspool.tile([128, 512], BF, tag="psl")
                    nc.vector.tensor_copy(psl_sb, psl)
                    for gi in range(9):   # 9 groups of 7 vocab m-tiles
                        vs = slice(gi * 7, gi * 7 + 7)
                        outf = vspool.tile([128, 7, 512], BF, tag="outf", bufs=1)
                        nc.vector.tensor_sub(outf, lg[:, vs, :], _bc(psl_sb, 1, 7))
                        nc.sync.dma_start(
                            out[gi * 7 * 128:(gi + 1) * 7 * 128, tsl]
                            .rearrange("(vm p) t -> p vm t", p=128), outf)
        else:
            # debug: dump hb as bf16 into the first D rows of out
            for kt in range(KT):
                dbg = spool.tile([128, T], BF, tag="outf")
                nc.vector.tensor_copy(dbg, hb[:, kt, :])
                nc.sync.dma_start(out[kt * 128:(kt + 1) * 128, :], dbg)
        _escope(_h)

    nsplit = split_sync_waits(nc)
    print(f"split_sync_waits: {nsplit} NOPs inserted")
    return nc


def _bf16(a):
    return np.asarray(a, dtype=ml_dtypes.bfloat16)


def _f8(a):
    return np.asarray(a, dtype=ml_dtypes.float8_e4m3)


def _pack_dr(w):
    """[1024, C] f32 -> [128, KT2, 2, C] fp8 with x64 prescale.
    Row r = 256*j + 128*i + p maps to [p, j, i, c]."""
    r = (np.asarray(w, np.float32) * WS).reshape(KT2, 2, 128, -1).transpose(2, 0, 1, 3)
    return _f8(np.ascontiguousarray(r))


def make_in_maps(x, tok_emb, pos_emb, wq, bq, wk, bk, wv, bv, wo, bo,
                 ln1_g, ln1_b, w1, b1, w2, b2, ln2_g, ln2_b, w_out, b_out):
    """Shard full inputs -> per-core input maps (host-reshaped fp8 weights)."""
    LE = wq.shape[0]
    per_r = []
    for r in range(TP):
        hs = slice(HL * r, HL * (r + 1))
        wqkv_r = np.concatenate(
            [
                wq[:, hs].transpose(0, 2, 1, 3).reshape(LE, D, HL * DH),
                wk[:, hs].transpose(0, 2, 1, 3).reshape(LE, D, HL * DH),
                wv[:, hs].transpose(0, 2, 1, 3).reshape(LE, D, HL * DH),
            ],
            axis=2,
        )
        bqkv_r = np.concatenate(
            [bq[:, hs].reshape(LE, -1), bk[:, hs].reshape(LE, -1),
             bv[:, hs].reshape(LE, -1)], axis=1,
        )
        fs = slice(FFL * r, FFL * (r + 1))
        vs = slice(VL * r, VL * (r + 1))
        wqkv8_r = np.stack([_pack_dr(wqkv_r[le]) for le in range(LE)])
        wo8_r = np.stack([_pack_dr(wo[le]) for le in range(LE)])
        w1r_r = np.stack([_bf16(np.ascontiguousarray(
            np.asarray(w1[le][:, fs], np.float32).reshape(KT, 128, FFL)
            .transpose(1, 0, 2))) for le in range(LE)])
        w2r_r = np.stack([_bf16(np.ascontiguousarray(
            np.asarray(w2[le][fs, :], np.float32).reshape(KT, 128, D)
            .transpose(1, 0, 2))) for le in range(LE)])
        wout_r = np.zeros((D, VLP), np.float32)
        wout_r[:, :VL] = w_out[:, vs]
        woutr_r = _bf16(np.ascontiguousarray(
            np.transpose(wout_r.reshape(KT, 128, NVM, 128), (2, 1, 0, 3))))
        bout_r = np.full((VLP,), -1e30, np.float32)
        bout_r[:VL] = b_out[vs]
        per_r.append(dict(
            wqkv8=wqkv8_r,
            bqkv=np.ascontiguousarray(bqkv_r, np.float32),
            wo8=wo8_r,
            bo=np.ascontiguousarray(bo, np.float32),
            ln1g=np.ascontiguousarray(ln1_g, np.float32),
            ln1b=np.ascontiguousarray(ln1_b, np.float32),
            w1r=w1r_r,
            b1=np.ascontiguousarray(b1[:, fs], np.float32),
            w2r=w2r_r,
            b2q4=np.ascontiguousarray(b2 / TP, np.float32),
            ln2g=np.ascontiguousarray(ln2_g, np.float32),
            ln2b=np.ascontiguousarray(ln2_b, np.float32),
            woutr=woutr_r,
            bout=bout_r,
        ))
    in_maps = []
    for c in range(8):
        g, r = c // TP, c % TP
        emb = np.asarray(tok_emb[x[g]] + pos_emb[:S], np.float32)   # [S, D]
        m = dict(per_r[r])
        embT = np.ascontiguousarray(emb.T)                          # [D, T]
        m["h0b"] = _bf16(embT)
        m["h0b8"] = _f8(np.ascontiguousarray(
            embT.reshape(KT, 128, T).transpose(1, 0, 2)))
        in_maps.append(m)
    return in_maps


_CACHED = {}


def kernel(**inputs):
    global FOLD_LN, FOLD_BIAS
    inputs = {k: np.asarray(v) for k, v in inputs.items()}
    fold_ln = (np.all(inputs["ln1_g"] == 1) and np.all(inputs["ln1_b"] == 0)
               and np.all(inputs["ln2_g"] == 1) and np.all(inputs["ln2_b"] == 0))
    fold_bias = (np.all(inputs["bo"] == 0) and np.all(inputs["b2"] == 0))
    key = ("nc", fold_ln, fold_bias)
    if key not in _CACHED:
        FOLD_LN, FOLD_BIAS = fold_ln, fold_bias
        _CACHED[key] = build_program()
    nc = _CACHED[key]
    in_maps = make_in_maps(**inputs)
    trace = os.environ.get("BASS_GPT_TRACE", "0") == "1"
    res = run_bass_kernel_spmd(
        nc, in_maps, core_ids=list(range(8)), trace=trace,
    )
    if trace:
        print(f"HW exec time: {res.exec_time_ns} ns")
        _CACHED["last_result"] = res
    results = res.results
    full = np.empty((B, S, V), np.float32)
    for c in range(8):
        g, r = c // TP, c % TP
        full[g, :, VL * r: VL * (r + 1)] = \
            np.asarray(results[c]["out"][:VL, :], np.float32).T
    return full


# revision 18
# speedup vs baseline: 1.4756x; 1.0958x over previous
"""GPT-style transformer forward on 8 Trainium2 NeuronCores.

Sharding: data-parallel over batch (2 groups of 4 cores), tensor-parallel
within each group (heads / FFN hidden / vocab columns split 4 ways).
Device activations are feature-major [feature, token] so all matmuls run
without transposes.

v3: fp8e4 DoubleRow matmuls (2x PE throughput) for QKV/out-proj/FFN/vocab
with host-prescaled weights (x64) compensated in the drains; exp(-ln(x))
reciprocals on the Activation engine (exact DVE reciprocal is 4.3us);
FFN residual add folded into the AllReduce inputs (hb/4 + b2/4); fp8
AllGather of attention head outputs; bf16 output tensor; elementwise work
rebalanced across DVE/Pool/Activation; full-layer weight prefetch via
double-buffered pools.
"""

import os
from contextlib import ExitStack

import numpy as np
import ml_dtypes

import concourse.bass as bass
import concourse.mybir as mybir
import concourse.tile as tile
from concourse.bass_utils import run_bass_kernel_spmd
from concourse.vector_clock import ScopedClock


def _drain_and_barrier(self, tick_clock, wait_clock):
    """The walrus build here encodes Drain/NoOp as TPB_CTRL with at most one
    sync-wait slot; Tile's stock tail attaches all outstanding waits to the
    Drain and fails codegen. Split the waits one-per-NOP instead."""
    nop_inst = self.nc.sync.nop(nofuse=True)
    wait_clock.add_sem_waits(nop_inst.ins, ScopedClock({None: tick_clock.global_clock}))
    si = nop_inst.ins.sync_info
    if si is not None and len(si.on_wait) > 1:
        waits = list(si.on_wait)
        nop_inst.ins.sync_info = mybir.SyncInfo(on_wait=waits[:1], on_update=list(si.on_update))
        for w in waits[1:]:
            n2 = self.nc.sync.nop(nofuse=True)
            n2.ins.sync_info = mybir.SyncInfo(on_wait=[w], on_update=[])
    self.nc.sync.drain()
    self.nc.all_engine_barrier()
    assert self.sems is not None
    popped = self.nc._tile_sem_poison_stack.pop()
    assert popped is self._sem_poison
    self.nc.clear_and_free_semaphores(list(self.sems.allocated().values()))
    self.nc.all_engine_barrier()


tile.TileContext._drain_and_barrier = _drain_and_barrier

_MAX_WAITS = 1  # this walrus build caps sync-waits per instruction


def split_sync_waits(nc):
    """Hoist excess on_wait entries onto same-engine NOPs inserted before the
    instruction (engine queues execute in program order, so semantics hold)."""
    n = 0
    for bb in nc.main_func.blocks:
        insts = bb.instructions
        new_list = []
        for inst in insts:
            si = getattr(inst, "sync_info", None)
            if si is not None and len(si.on_wait) > _MAX_WAITS:
                waits = list(si.on_wait)
                for w in waits[:-_MAX_WAITS]:
                    n += 1
                    new_list.append(mybir.InstNoOp(
                        name=f"{inst.name}-sw{n}",
                        sync_info=mybir.SyncInfo(on_wait=[w], on_update=[]),
                        bass_nofuse=True,
                        engine=inst.engine,
                    ))
                inst.sync_info = mybir.SyncInfo(
                    on_wait=waits[-_MAX_WAITS:], on_update=list(si.on_update)
                )
            new_list.append(inst)
        if len(new_list) != len(insts):
            bb.instructions[:] = new_list
    return n


# Model dims (hardcoded per problem spec)
L_FULL, H, D, V, SMAX = 8, 16, 1024, 32000, 1024
DH = D // H          # 64
FF = 4 * D           # 4096
B, S = 2, 1024
T = S                # tokens per group (one batch element per group)
TP = 4               # tensor-parallel degree within a group
HL = H // TP         # 4 local heads
FFL = FF // TP       # 1024 local FFN cols
VL = V // TP         # 8000 local vocab cols
VLP = 8064           # padded to 63*128
NVM = VLP // 128     # 63 vocab m-tiles
EPS = 1e-5
KT = D // 128        # 8 k-tiles over model dim
KT2 = KT // 2        # 4 fp8 DoubleRow k-pairs
NB = T // 512        # 2 token blocks of 512

WS = 64.0            # fp8 weight prescale
WSI = 1.0 / WS

BF = mybir.dt.bfloat16
F32 = mybir.dt.float32
F8 = mybir.dt.float8e4
AF = mybir.ActivationFunctionType
ALU = mybir.AluOpType
DR = mybir.MatmulPerfMode.DoubleRow

RG = [[0, 1, 2, 3], [4, 5, 6, 7]]

N_LAYERS = int(os.environ.get("BASS_GPT_LAYERS", str(L_FULL)))
SKIP_FINAL = os.environ.get("BASS_GPT_SKIP_FINAL", "0") == "1"

# Set from the actual inputs before build: when LN gains are all-ones /
# biases all-zero (true for this model family), the g/b passes and bias
# adds are dropped and scale-invariance folds the fp8 compensation into
# the residual adds.
FOLD_LN = True
FOLD_BIAS = True


def _r2(ap):
    """[ (kt p) n ] -> [p kt n] view of a DRAM 2-D tensor (p=128)."""
    return ap.rearrange("(kt p) n -> p kt n", p=128)


def _bc(ap, dim, n):
    """Insert a stride-0 broadcast dim of size n at position `dim`."""
    newap = [list(d) for d in ap.ap]
    newap.insert(dim, [0, n])
    return bass.AP(ap.tensor, ap.offset, newap)


def build_program():
    nc = bass.Bass("TRN2")

    # ---- DRAM parameters (per-core shards, host-reshaped) ----
    h0b = nc.declare_dram_parameter("h0b", [D, T], BF, isOutput=False)
    h0b8 = nc.declare_dram_parameter("h0b8", [128, KT, T], F8, isOutput=False)
    wqkv8 = nc.declare_dram_parameter("wqkv8", [N_LAYERS, 128, KT2, 2, 3 * HL * DH], F8, isOutput=False)
    bqkv = nc.declare_dram_parameter("bqkv", [N_LAYERS, 3 * HL * DH], F32, isOutput=False)
    wo8 = nc.declare_dram_parameter("wo8", [N_LAYERS, 128, KT2, 2, D], F8, isOutput=False)
    bo = nc.declare_dram_parameter("bo", [N_LAYERS, D], F32, isOutput=False)
    ln1g = nc.declare_dram_parameter("ln1g", [N_LAYERS, D], F32, isOutput=False)
    ln1b = nc.declare_dram_parameter("ln1b", [N_LAYERS, D], F32, isOutput=False)
    w1r = nc.declare_dram_parameter("w1r", [N_LAYERS, 128, KT, FFL], BF, isOutput=False)
    b1 = nc.declare_dram_parameter("b1", [N_LAYERS, FFL], F32, isOutput=False)
    w2r = nc.declare_dram_parameter("w2r", [N_LAYERS, 128, KT, D], BF, isOutput=False)
    b2q4 = nc.declare_dram_parameter("b2q4", [N_LAYERS, D], F32, isOutput=False)
    ln2g = nc.declare_dram_parameter("ln2g", [N_LAYERS, D], F32, isOutput=False)
    ln2b = nc.declare_dram_parameter("ln2b", [N_LAYERS, D], F32, isOutput=False)
    woutr = nc.declare_dram_parameter("woutr", [NVM, 128, KT, 128], BF, isOutput=False)
    bout = nc.declare_dram_parameter("bout", [VLP], F32, isOutput=False)
    out = nc.declare_dram_parameter("out", [VLP, T], BF, isOutput=True)

    with ExitStack() as ctx:
        tc = ctx.enter_context(tile.TileContext(nc))

        def _scope(name):
            sid, _ = nc.enter_named_scope(name, False)
            return (name, sid)

        def _escope(h):
            nc.leave_named_scope(h[0], h[1], False)

        # ---- outer pools (live whole program) ----
        const = ctx.enter_context(tc.tile_pool(name="const", bufs=1))
        hpool = ctx.enter_context(tc.tile_pool(name="hpool", bufs=1))
        spool = ctx.enter_context(tc.tile_pool(name="spool", bufs=2))
        rpool = ctx.enter_context(tc.tile_pool(name="rpool", bufs=2))
        mm_psum = ctx.enter_context(tc.tile_pool(name="mm_psum", bufs=3, space="PSUM"))
        bc_psum = ctx.enter_context(tc.tile_pool(name="bc_psum", bufs=1, space="PSUM"))
        dram = ctx.enter_context(tc.tile_pool(name="dram", bufs=2, space="DRAM"))

        # ---- constants ----
        ones_k = const.tile([128, 1], BF)       # lhsT for partition-sum (M=1)
        nc.vector.memset(ones_k, 1.0)
        ones128 = const.tile([128, 128], BF)    # lhsT for bcast partition-sum (M=128)
        nc.vector.memset(ones128, 1.0)
        ones_m = const.tile([1, 128], BF)       # lhsT for broadcast (K=1, M=128)
        nc.vector.memset(ones_m, 1.0)
        eps128 = const.tile([128, 1], F32)
        nc.vector.memset(eps128, float(EPS))
        # causal keep-masks: variant j keeps where t1f - t2p - 128*j >= 0
        maskq = const.tile([128, 4, 512], BF)
        nc.gpsimd.memset(maskq, 1.0)
        for j in range(4):
            nc.gpsimd.affine_select(
                out=maskq[:, j, :], in_=maskq[:, j, :],
                compare_op=ALU.is_ge, fill=0.0,
                base=-128 * j, pattern=[[1, 512]], channel_multiplier=-1,
            )

        # ---- persistent activation state ----
        hb = hpool.tile([128, KT, T], BF)       # residual stream (feature-major)
        nc.sync.dma_start(hb, _r2(h0b))
        hb8 = hpool.tile([128, KT, T], F8)      # fp8 copy for DR matmul rhs
        nc.sync.dma_start(hb8, h0b8[:])

        with ExitStack() as lctx:
            apool = lctx.enter_context(tc.tile_pool(name="apool", bufs=1))
            xpool = lctx.enter_context(tc.tile_pool(name="xpool", bufs=1))
            fpool = lctx.enter_context(tc.tile_pool(name="fpool", bufs=1))
            epool = lctx.enter_context(tc.tile_pool(name="epool", bufs=2))
            wq_pool = lctx.enter_context(tc.tile_pool(name="wq_pool", bufs=2))
            wo_pool = lctx.enter_context(tc.tile_pool(name="wo_pool", bufs=2))
            w1_pool = lctx.enter_context(tc.tile_pool(name="w1_pool", bufs=1))
            w2_pool = lctx.enter_context(tc.tile_pool(name="w2_pool", bufs=1))
            agp = lctx.enter_context(tc.tile_pool(name="agp", bufs=2))
            oarp = lctx.enter_context(tc.tile_pool(name="oarp", bufs=1))
            xsqb = lctx.enter_context(tc.tile_pool(name="xsqb", bufs=1))
            bpool = lctx.enter_context(tc.tile_pool(name="bpool", bufs=2))
            stp = lctx.enter_context(tc.tile_pool(name="stp", bufs=1))
            o_psum = lctx.enter_context(tc.tile_pool(name="o_psum", bufs=2, space="PSUM"))
            st_psum = lctx.enter_context(tc.tile_pool(name="st_psum", bufs=1, space="PSUM"))

            qk_sb = apool.tile([128, 2, 2, T], BF)   # [part, q/k, head-pair, t]
            vaug = apool.tile([128, KT, HL, 65], BF)  # token-major V + ones col
            oT8 = apool.tile([128, 2, T], F8)        # attn head outputs (feature-major)
            x1b = xpool.tile([128, KT, T], BF)       # pre-LN1 accumulator
            nc.vector.memset(vaug[:, :, :, 64:65], 1.0)

            def ln_stats(xsrc, tsl, tag):
                """Broadcast stats for tokens tsl of xsrc [128,KT,*] bf16.
                Returns (mu_bc, rp_bc) SBUF [128,512]; rp via exp(-.5 ln).
                Scale-invariant consumers let xsrc carry any uniform scale."""
                ps_s1 = st_psum.tile([128, 512], F32, tag="st1")
                ps_s2 = st_psum.tile([128, 512], F32, tag="st2")
                xsq = xsqb.tile([128, KT, 512], BF, tag="xsq")
                nc.vector.tensor_mul(xsq[:, 0:5, :], xsrc[:, 0:5, tsl], xsrc[:, 0:5, tsl])
                nc.scalar.activation(xsq[:, 5:8, :], xsrc[:, 5:8, tsl], AF.Square)
                for kt in range(KT):
                    nc.tensor.matmul(ps_s1, ones128, xsrc[:, kt, tsl],
                                     start=(kt == 0), stop=(kt == KT - 1))
                for kt in range(KT):
                    nc.tensor.matmul(ps_s2, ones128, xsq[:, kt, :],
                                     start=(kt == 0), stop=(kt == KT - 1))
                mu_bc = stp.tile([128, 512], BF, tag=f"mu{tag}")
                nc.vector.tensor_scalar(out=mu_bc, in0=ps_s1, scalar1=1.0 / D,
                                        scalar2=None, op0=ALU.mult)
                m1sq = spool.tile([128, 512], BF, tag="m1sq")
                nc.vector.tensor_mul(m1sq, mu_bc, mu_bc)
                u = spool.tile([128, 512], BF, tag="uvar")
                nc.vector.scalar_tensor_tensor(
                    out=u, in0=ps_s2, scalar=1.0 / D, in1=m1sq,
                    op0=ALU.mult, op1=ALU.subtract)
                lnu = spool.tile([128, 512], BF, tag="lnu")
                nc.scalar.activation(lnu, u, AF.Ln, bias=eps128[:, 0:1])
                rp_bc = stp.tile([128, 512], BF, tag=f"rp{tag}")
                nc.scalar.activation(rp_bc, lnu, AF.Exp, scale=-0.5)
                return mu_bc, rp_bc

            def ln_apply(xsrc, tsl, mu_bc, rp_bc, g_sb, b_sb, hb_tsl=None):
                """hb[:, :, hb_tsl] = (xsrc - mu)*rp*g + b ; hb8 = fp8(hb).
                Batched over kt with stride-0 broadcast of mu/rp; when
                FOLD_LN (g==1, b==0) the g/b passes are dropped."""
                if hb_tsl is None:
                    hb_tsl = tsl
                xm = xsqb.tile([128, KT, 512], BF, tag="xmb")
                nc.vector.tensor_sub(xm[:, 0:5, :], xsrc[:, 0:5, tsl], _bc(mu_bc, 1, 5))
                nc.vector.tensor_sub(xm[:, 5:8, :], xsrc[:, 5:8, tsl], _bc(mu_bc, 1, 3))
                if FOLD_LN:
                    nc.vector.tensor_mul(hb[:, 0:5, hb_tsl], xm[:, 0:5, :], _bc(rp_bc, 1, 5))
                    nc.vector.tensor_mul(hb[:, 5:8, hb_tsl], xm[:, 5:8, :], _bc(rp_bc, 1, 3))
                else:
                    nc.vector.tensor_mul(xm[:, 0:5, :], xm[:, 0:5, :], _bc(rp_bc, 1, 5))
                    nc.gpsimd.tensor_mul(xm[:, 5:8, :], xm[:, 5:8, :], _bc(rp_bc, 1, 3))
                    g3 = _bc(g_sb[:, 0:KT], 2, 512)
                    b3 = _bc(b_sb[:, 0:KT], 2, 512)
                    nc.vector.tensor_mul(xm, xm, g3)
                    nc.vector.tensor_add(hb[:, :, hb_tsl], xm, b3)
                nc.vector.tensor_copy(hb8[:, 0:5, hb_tsl], hb[:, 0:5, hb_tsl])
                nc.scalar.activation(hb8[:, 5:8, hb_tsl], hb[:, 5:8, hb_tsl], AF.Copy)

            def phase_qkv(l, wqkv_sb, bqkv_sb, blk):
                tsl = slice(blk * 512, (blk + 1) * 512)
                for io in range(2):        # 0=q, 1=k  (feature-major out)
                    for mt in range(2):    # head pair
                        mcol = (io * 2 + mt) * 128
                        ps = mm_psum.tile([128, 512], F32, tag="mm")
                        for j in range(KT2):
                            nc.tensor.matmul(
                                ps, wqkv_sb[:, j, :, mcol:mcol + 128],
                                hb8[:, 2 * j:2 * j + 2, tsl],
                                start=(j == 0), stop=(j == KT2 - 1), perf_mode=DR)
                        nc.vector.tensor_scalar(
                            out=qk_sb[:, io, mt, tsl], in0=ps,
                            scalar1=WSI, scalar2=bqkv_sb[:, io * 2 + mt:io * 2 + mt + 1],
                            op0=ALU.mult, op1=ALU.add)
                for tm in range(4 * blk, 4 * (blk + 1)):   # v, token-major
                    ps = mm_psum.tile([128, 256], F32, tag="mm")
                    for j in range(KT2):
                        nc.tensor.matmul(
                            ps, hb8[:, 2 * j:2 * j + 2, tm * 128:(tm + 1) * 128],
                            wqkv_sb[:, j, :, 512:768],
                            start=(j == 0), stop=(j == KT2 - 1), perf_mode=DR)
                    nc.vector.tensor_scalar(
                        out=vaug[:, tm, :, 0:64],
                        in0=ps.rearrange("p (h e) -> p h e", h=HL),
                        scalar1=WSI, scalar2=None, op0=ALU.mult)

            def phase_attn(l, bqkv_sb, blk):
                t1sl = slice(blk * 512, (blk + 1) * 512)
                t2max = 4 * (blk + 1)
                for h in range(HL):
                    prow = slice(64 * (h % 2), 64 * (h % 2) + 64)
                    hm = h // 2
                    et = epool.tile([128, KT, 512], BF, tag="eT")
                    for t2t in range(t2max):
                        ps = mm_psum.tile([128, 512], F32, tag="mm")
                        nc.tensor.matmul(
                            ps,
                            qk_sb[prow, 1, hm, t2t * 128:(t2t + 1) * 128],
                            qk_sb[prow, 0, hm, t1sl],
                            start=True, stop=True)
                        nc.scalar.activation(et[:, t2t, :], ps, AF.Exp, scale=0.125)
                    dg = slice(4 * blk, 4 * blk + 4)
                    nc.vector.tensor_mul(et[:, dg, :], et[:, dg, :], maskq)
                    ps_o = o_psum.tile([65, 512], F32, tag="o")
                    for t2t in range(t2max):
                        nc.tensor.matmul(
                            ps_o, vaug[:, t2t, h, :], et[:, t2t, :],
                            start=(t2t == 0), stop=(t2t == t2max - 1))
                    # 1/Z via exp(-ln(Z)) on the Activation engine
                    lnz = rpool.tile([1, 512], F32, tag="lnz")
                    nc.scalar.activation(lnz, ps_o[64:65, :], AF.Ln)
                    rec = rpool.tile([1, 512], BF, tag="rec")
                    nc.scalar.activation(rec, lnz, AF.Exp, scale=-1.0)
                    ps_b = bc_psum.tile([64, 512], F32, tag="bc")
                    nc.tensor.matmul(ps_b, ones_m[:, 0:64], rec, start=True, stop=True)
                    psb_sb = spool.tile([64, 512], BF, tag="psb")
                    nc.vector.tensor_copy(psb_sb, ps_b)
                    tmp = spool.tile([64, 512], BF, tag="otmp")
                    nc.vector.tensor_mul(tmp, ps_o[0:64, :], psb_sb)
                    nc.vector.tensor_scalar(
                        out=oT8[prow, hm, t1sl], in0=tmp,
                        scalar1=bqkv_sb[prow, 4 + hm:5 + hm], scalar2=None,
                        op0=ALU.add)

            def stage_ag(blk):
                tsl = slice(blk * 512, (blk + 1) * 512)
                ag_in = dram.tile([2 * 128, 512], F8, tag="agin")
                for pt in range(2):
                    nc.sync.dma_start(ag_in[pt * 128:(pt + 1) * 128, :], oT8[:, pt, tsl])
                ag_out = dram.tile([D, 512], F8, tag="agout")
                nc.gpsimd.collective_compute(
                    "AllGather", ALU.bypass, replica_groups=RG,
                    ins=[ag_in.opt()], outs=[ag_out.opt()])
                return ag_out

            def phase_oproj(l, wo_sb, bo_sb, ag_out, blk):
                """Full out-proj from gathered head outputs; x1b = oproj*WSI + bo + hb."""
                tsl = slice(blk * 512, (blk + 1) * 512)
                agh = agp.tile([128, KT, 512], F8, tag="agh")
                nc.sync.dma_start(agh, _r2(ag_out))
                for mt in range(KT):
                    ps = mm_psum.tile([128, 512], F32, tag="mm")
                    for j in range(KT2):
                        nc.tensor.matmul(
                            ps, wo_sb[:, j, :, mt * 128:(mt + 1) * 128],
                            agh[:, 2 * j:2 * j + 2, :],
                            start=(j == 0), stop=(j == KT2 - 1), perf_mode=DR)
                    if FOLD_BIAS:
                        # x1b = WS*(oproj + hb); LN1 is scale-invariant
                        nc.vector.scalar_tensor_tensor(
                            out=x1b[:, mt, tsl], in0=hb[:, mt, tsl], scalar=WS,
                            in1=ps, op0=ALU.mult, op1=ALU.add)
                    else:
                        op = spool.tile([128, 512], BF, tag="ob")
                        nc.vector.tensor_scalar(
                            out=op, in0=ps, scalar1=WSI, scalar2=bo_sb[:, mt:mt + 1],
                            op0=ALU.mult, op1=ALU.add)
                        if mt % 2 == 0:
                            nc.gpsimd.tensor_add(x1b[:, mt, tsl], op, hb[:, mt, tsl])
                        else:
                            nc.vector.tensor_add(x1b[:, mt, tsl], op, hb[:, mt, tsl])

            def phase_ffn(l, w1_sb, w2_sb, b1_sb, b2q4_sb, blk):
                tsl = slice(blk * 512, (blk + 1) * 512)
                f1 = fpool.tile([128, KT, 512], BF, tag="f1")
                for mt in range(KT):
                    ps = mm_psum.tile([128, 512], F32, tag="mm")
                    for kt in range(KT):
                        nc.tensor.matmul(
                            ps, w1_sb[:, kt, mt * 128:(mt + 1) * 128],
                            hb[:, kt, tsl],
                            start=(kt == 0), stop=(kt == KT - 1))
                    nc.scalar.activation(f1[:, mt, :], ps, AF.Relu,
                                         bias=b1_sb[:, mt:mt + 1])
                # split AllReduce into kt halves so LN2 stats start early
                ar_outs = []
                for half in range(2):
                    ar_in = dram.tile([D // 2, 512], BF, tag=f"arin{half}")
                    for mi in range(4):
                        mt = half * 4 + mi
                        ps = mm_psum.tile([128, 512], F32, tag="mm")
                        for kt in range(KT):
                            nc.tensor.matmul(
                                ps, w2_sb[:, kt, mt * 128:(mt + 1) * 128],
                                f1[:, kt, :],
                                start=(kt == 0), stop=(kt == KT - 1))
                        if FOLD_BIAS:
                            # ob = hb/4 + ff_partial -> AR yields x2 exactly
                            ob = spool.tile([128, 512], BF, tag="ob")
                            nc.vector.scalar_tensor_tensor(
                                out=ob, in0=hb[:, mt, tsl], scalar=0.25,
                                in1=ps, op0=ALU.mult, op1=ALU.add)
                        else:
                            hq = spool.tile([128, 512], BF, tag="xm")
                            nc.vector.tensor_scalar(
                                out=hq, in0=hb[:, mt, tsl], scalar1=0.25,
                                scalar2=b2q4_sb[:, mt:mt + 1], op0=ALU.mult, op1=ALU.add)
                            ob = spool.tile([128, 512], BF, tag="ob")
                            nc.vector.scalar_tensor_tensor(
                                out=ob, in0=ps, scalar=WSI, in1=hq,
                                op0=ALU.mult, op1=ALU.add)
                        nc.sync.dma_start(ar_in[mi * 128:(mi + 1) * 128, :], ob)
                    ar_out = dram.tile([D // 2, 512], BF, tag=f"arout{half}")
                    nc.gpsimd.collective_compute(
                        "AllReduce", ALU.add, replica_groups=RG,
                        ins=[ar_in.opt()], outs=[ar_out.opt()])
                    ar_outs.append(ar_out)
                return ar_outs

            def phase_ln2(l, ar_outs, g_sb, b_sb, blk):
                """AR output already includes the (scaled) residual; LN2 -> hb."""
                tsl = slice(blk * 512, (blk + 1) * 512)
                oar = oarp.tile([128, KT, 512], BF, tag="oar")
                nc.sync.dma_start(oar[:, 0:4, :], _r2(ar_outs[0]))
                nc.sync.dma_start(oar[:, 4:8, :], _r2(ar_outs[1]))
                mu, rp = ln_stats(oar, slice(0, 512), "b")
                ln_apply(oar, slice(0, 512), mu, rp, g_sb, b_sb, hb_tsl=tsl)

            # ---- layer loop, software-pipelined across the block dim ----
            pend_f1 = None   # deferred (ar_out1, g2_sb, bb2_sb) from prev layer
            for l in range(N_LAYERS):
                _h = _scope(f"L{l}.wload")
                wqkv_sb = wq_pool.tile([128, KT2, 2, 768], F8, tag="wqkv")
                nc.sync.dma_start(wqkv_sb, wqkv8[l])
                wo_sb = wo_pool.tile([128, KT2, 2, D], F8, tag="wo")
                nc.sync.dma_start(wo_sb, wo8[l])
                w1_sb = w1_pool.tile([128, KT, FFL], BF, tag="w1")
                nc.sync.dma_start(w1_sb, w1r[l])
                w2_sb = w2_pool.tile([128, KT, D], BF, tag="w2")
                nc.sync.dma_start(w2_sb, w2r[l])
                bqkv_sb = bpool.tile([128, 6], F32, tag="bqkv")
                nc.sync.dma_start(bqkv_sb, bqkv[l].rearrange("(m p) -> p m", p=128))
                bo_sb = bpool.tile([128, KT], F32, tag="bo")
                nc.sync.dma_start(bo_sb, bo[l].rearrange("(m p) -> p m", p=128))
                g1_sb = bpool.tile([128, KT], F32, tag="g1")
                nc.sync.dma_start(g1_sb, ln1g[l].rearrange("(m p) -> p m", p=128))
                bb1_sb = bpool.tile([128, KT], F32, tag="bb1")
                nc.sync.dma_start(bb1_sb, ln1b[l].rearrange("(m p) -> p m", p=128))
                b1_sb = bpool.tile([128, KT], F32, tag="b1")
                nc.sync.dma_start(b1_sb, b1[l].rearrange("(m p) -> p m", p=128))
                b2q4_sb = bpool.tile([128, KT], F32, tag="b2")
                nc.sync.dma_start(b2q4_sb, b2q4[l].rearrange("(m p) -> p m", p=128))
                g2_sb = bpool.tile([128, KT], F32, tag="g2")
                nc.sync.dma_start(g2_sb, ln2g[l].rearrange("(m p) -> p m", p=128))
                bb2_sb = bpool.tile([128, KT], F32, tag="bb2")
                nc.sync.dma_start(bb2_sb, ln2b[l].rearrange("(m p) -> p m", p=128))
                _escope(_h)

                _h = _scope(f"L{l}.ab0")
                phase_qkv(l, wqkv_sb, bqkv_sb, 0)
                phase_attn(l, bqkv_sb, 0)
                ag0 = stage_ag(0)
                _escope(_h)

                if pend_f1 is not None:
                    _h = _scope(f"L{l}.f1prev")
                    phase_ln2(l - 1, *pend_f1, 1)
                    _escope(_h)
                    pend_f1 = None

                _h = _scope(f"L{l}.ab1")
                phase_qkv(l, wqkv_sb, bqkv_sb, 1)
                phase_attn(l, bqkv_sb, 1)
                ag1 = stage_ag(1)
                _escope(_h)

                _h = _scope(f"L{l}.d0")
                phase_oproj(l, wo_sb, bo_sb, ag0, 0)
                mu0, rp0 = ln_stats(x1b, slice(0, 512), "a")
                _escope(_h)
                _h = _scope(f"L{l}.d1")
                phase_oproj(l, wo_sb, bo_sb, ag1, 1)
                _escope(_h)
                _h = _scope(f"L{l}.e0")
                ln_apply(x1b, slice(0, 512), mu0, rp0, g1_sb, bb1_sb)
                ar0 = phase_ffn(l, w1_sb, w2_sb, b1_sb, b2q4_sb, 0)
                _escope(_h)
                _h = _scope(f"L{l}.e1")
                mu1, rp1 = ln_stats(x1b, slice(512, 1024), "a")
                ln_apply(x1b, slice(512, 1024), mu1, rp1, g1_sb, bb1_sb)
                ar1 = phase_ffn(l, w1_sb, w2_sb, b1_sb, b2q4_sb, 1)
                _escope(_h)
                _h = _scope(f"L{l}.f0")
                phase_ln2(l, ar0, g2_sb, bb2_sb, 0)
                _escope(_h)
                pend_f1 = (ar1, g2_sb, bb2_sb)

            _h = _scope("flast")
            phase_ln2(N_LAYERS - 1, *pend_f1, 1)
            _escope(_h)

        # ---- vocab projection + log-softmax (layer pools now closed) ----
        _h = _scope("vocab")
        if not SKIP_FINAL:
            with ExitStack() as vctx:
                lgp = vctx.enter_context(tc.tile_pool(name="lgp", bufs=2))
                vwch = vctx.enter_context(tc.tile_pool(name="vwch", bufs=6))
                vspool = vctx.enter_context(tc.tile_pool(name="vspool", bufs=2))
                vdram = vctx.enter_context(tc.tile_pool(name="vdram", bufs=2, space="DRAM"))
                va_psum = vctx.enter_context(tc.tile_pool(name="va_psum", bufs=2, space="PSUM"))

                bout_sb = const.tile([128, NVM], F32)
                nc.sync.dma_start(bout_sb, bout.rearrange("(m p) -> p m", p=128))
                for th in range(2):
                    tsl = slice(th * 512, (th + 1) * 512)
                    lg = lgp.tile([128, NVM, 512], BF, tag="lg")
                    ps_acc = va_psum.tile([1, 512], F32, tag="vacc")
                    for vm in range(NVM):
                        wv_sb = vwch.tile([128, KT, 128], BF, tag="vw")
                        nc.sync.dma_start(wv_sb, woutr[vm])
                        ps = mm_psum.tile([128, 512], F32, tag="mm")
                        for kt in range(KT):
                            nc.tensor.matmul(ps, wv_sb[:, kt, :], hb[:, kt, tsl],
                                             start=(kt == 0), stop=(kt == KT - 1))
                        if vm % 2 == 0:
                            nc.vector.tensor_scalar(
                                out=lg[:, vm, :], in0=ps, scalar1=bout_sb[:, vm:vm + 1],
                                scalar2=None, op0=ALU.add)
                        else:
                            nc.scalar.activation(lg[:, vm, :], ps, AF.Identity,
                                                 bias=bout_sb[:, vm:vm + 1])
                        eb = vspool.tile([128, 512], BF, tag="eb")
                        nc.scalar.activation(eb, ps, AF.Exp,
                                             bias=bout_sb[:, vm:vm + 1])
                        nc.tensor.matmul(
                            ps_acc, ones_k, eb,
                            start=(vm == 0), stop=(vm == NVM - 1), skip_group_check=True)
                    se_row = rpool.tile([1, 512], F32, tag="serow")
                    nc.vector.tensor_copy(se_row, ps_acc)
                    se_in = vdram.tile([1, 512], F32, tag="sein")
                    nc.sync.dma_start(se_in, se_row)
                    se_out = vdram.tile([1, 512], F32, tag="seout")
                    nc.gpsimd.collective_compute(
                        "AllReduce", ALU.add, replica_groups=RG,
                        ins=[se_in.opt()], outs=[se_out.opt()])
                    se_sb = rpool.tile([1, 512], F32, tag="sesb")
                    nc.sync.dma_start(se_sb, se_out)
                    lr = rpool.tile([1, 512], BF, tag="lr")
                    nc.scalar.activation(lr, se_sb, AF.Ln)
                    psl = bc_psum.tile([128, 512], F32, tag="bc")
                    nc.tensor.matmul(psl, ones_m, lr, start=True, stop=True)
                    psl_sb = vspool.tile([128, 512], BF, tag="psl")
                    nc.vector.tensor_copy(psl_sb, psl)
                    for gi in range(9):   # 9 groups of 7 vocab m-tiles
                        vs = slice(gi * 7, gi * 7 + 7)
                        outf = vspool.tile([128, 7, 512], BF, tag="outf", bufs=1)
                        nc.vector.tensor_sub(outf, lg[:, vs, :], _bc(psl_sb, 1, 7))
                        nc.sync.dma_start(
                            out[gi * 7 * 128:(gi + 1) * 7 * 128, tsl]
                            .rearrange("(vm p) t -> p vm t", p=128), outf)
        else:
            # debug: dump hb as bf16 into the first D rows of out
            for kt in range(KT):
                dbg = spool.tile([128, T], BF, tag="outf")
                nc.vector.tensor_copy(dbg, hb[:, kt, :])
                nc.sync.dma_start(out[kt * 128:(kt + 1) * 128, :], dbg)
        _escope(_h)

    nsplit = split_sync_waits(nc)
    print(f"split_sync_waits: {nsplit} NOPs inserted")
    return nc


def _bf16(a):
    return np.asarray(a, dtype=ml_dtypes.bfloat16)


def _f8(a):
    return np.asarray(a, dtype=ml_dtypes.float8_e4m3)


def _pack_dr(w):
    """[1024, C] f32 -> [128, KT2, 2, C] fp8 with x64 prescale.
    Row r = 256*j + 128*i + p maps to [p, j, i, c]."""
    r = (np.asarray(w, np.float32) * WS).reshape(KT2, 2, 128, -1).transpose(2, 0, 1, 3)
    return _f8(np.ascontiguousarray(r))


def make_in_maps(x, tok_emb, pos_emb, wq, bq, wk, bk, wv, bv, wo, bo,
                 ln1_g, ln1_b, w1, b1, w2, b2, ln2_g, ln2_b, w_out, b_out):
    """Shard full inputs -> per-core input maps (host-reshaped fp8 weights)."""
    LE = wq.shape[0]
    per_r = []
    for r in range(TP):
        hs = slice(HL * r, HL * (r + 1))
        wqkv_r = np.concatenate(
            [
                wq[:, hs].transpose(0, 2, 1, 3).reshape(LE, D, HL * DH),
                wk[:, hs].transpose(0, 2, 1, 3).reshape(LE, D, HL * DH),
                wv[:, hs].transpose(0, 2, 1, 3).reshape(LE, D, HL * DH),
            ],
            axis=2,
        )
        bqkv_r = np.concatenate(
            [bq[:, hs].reshape(LE, -1), bk[:, hs].reshape(LE, -1),
             bv[:, hs].reshape(LE, -1)], axis=1,
        )
        fs = slice(FFL * r, FFL * (r + 1))
        vs = slice(VL * r, VL * (r + 1))
        wqkv8_r = np.stack([_pack_dr(wqkv_r[le]) for le in range(LE)])
        wo8_r = np.stack([_pack_dr(wo[le]) for le in range(LE)])
        w1r_r = np.stack([_bf16(np.ascontiguousarray(
            np.asarray(w1[le][:, fs], np.float32).reshape(KT, 128, FFL)
            .transpose(1, 0, 2))) for le in range(LE)])
        w2r_r = np.stack([_bf16(np.ascontiguousarray(
            np.asarray(w2[le][fs, :], np.float32).reshape(KT, 128, D)
            .transpose(1, 0, 2))) for le in range(LE)])
        wout_r = np.zeros((D, VLP), np.float32)
        wout_r[:, :VL] = w_out[:, vs]
        woutr_r = _bf16(np.ascontiguousarray(
            np.transpose(wout_r.reshape(KT, 128, NVM, 128), (2, 1, 0, 3))))
        bout_r = np.full((VLP,), -1e30, np.float32)
        bout_r[:VL] = b_out[vs]
        per_r.append(dict(
            wqkv8=wqkv8_r,
            bqkv=np.ascontiguousarray(bqkv_r, np.float32),
            wo8=wo8_r,
            bo=np.ascontiguousarray(bo, np.float32),
            ln1g=np.ascontiguousarray(ln1_g, np.float32),
            ln1b=np.ascontiguousarray(ln1_b, np.float32),
            w1r=w1r_r,
            b1=np.ascontiguousarray(b1[:, fs], np.float32),
            w2r=w2r_r,
            b2q4=np.ascontiguousarray(b2 / TP, np.float32),
            ln2g=np.ascontiguousarray(ln2_g, np.float32),
            ln2b=np.ascontiguousarray(ln2_b, np.float32),
            woutr=woutr_r,
            bout=bout_r,
        ))
    in_maps = []
    for c in range(8):
        g, r = c // TP, c % TP
        emb = np.asarray(tok_emb[x[g]] + pos_emb[:S], np.float32)   # [S, D]
        m = dict(per_r[r])
        embT = np.ascontiguousarray(emb.T)                          # [D, T]
        m["h0b"] = _bf16(embT)
        m["h0b8"] = _f8(np.ascontiguousarray(
            embT.reshape(KT, 128, T).transpose(1, 0, 2)))
        in_maps.append(m)
    return in_maps


_CACHED = {}


def kernel(**inputs):
    global FOLD_LN, FOLD_BIAS
    inputs = {k: np.asarray(v) for k, v in inputs.items()}
    fold_ln = (np.all(inputs["ln1_g"] == 1) and np.all(inputs["ln1_b"] == 0)
               and np.all(inputs["ln2_g"] == 1) and np.all(inputs["ln2_b"] == 0))
    fold_bias = (np.all(inputs["bo"] == 0) and np.all(inputs["b2"] == 0))
    key = ("nc", fold_ln, fold_bias)
    if key not in _CACHED:
        FOLD_LN, FOLD_BIAS = fold_ln, fold_bias
        _CACHED[key] = build_program()
    nc = _CACHED[key]
    in_maps = make_in_maps(**inputs)
    trace = os.environ.get("BASS_GPT_TRACE", "0") == "1"
    res = run_bass_kernel_spmd(
        nc, in_maps, core_ids=list(range(8)), trace=trace,
    )
    if trace:
        print(f"HW exec time: {res.exec_time_ns} ns")
        _CACHED["last_result"] = res
    results = res.results
    full = np.empty((B, S, V), np.float32)
    for c in range(8):
        g, r = c // TP, c % TP
        full[g, :, VL * r: VL * (r + 1)] = \
            np.asarray(results[c]["out"][:VL, :], np.float32).T
    return full


# revision 19
# speedup vs baseline: 1.5047x; 1.0197x over previous
"""GPT-style transformer forward on 8 Trainium2 NeuronCores.

Sharding: data-parallel over batch (2 groups of 4 cores), tensor-parallel
within each group (heads / FFN hidden / vocab columns split 4 ways).
Device activations are feature-major [feature, token] so all matmuls run
without transposes.

v3: fp8e4 DoubleRow matmuls (2x PE throughput) for QKV/out-proj/FFN/vocab
with host-prescaled weights (x64) compensated in the drains; exp(-ln(x))
reciprocals on the Activation engine (exact DVE reciprocal is 4.3us);
FFN residual add folded into the AllReduce inputs (hb/4 + b2/4); fp8
AllGather of attention head outputs; bf16 output tensor; elementwise work
rebalanced across DVE/Pool/Activation; full-layer weight prefetch via
double-buffered pools.
"""

import os
from contextlib import ExitStack

import numpy as np
import ml_dtypes

import concourse.bass as bass
import concourse.mybir as mybir
import concourse.tile as tile
from concourse.bass_utils import run_bass_kernel_spmd
from concourse.vector_clock import ScopedClock


def _drain_and_barrier(self, tick_clock, wait_clock):
    """The walrus build here encodes Drain/NoOp as TPB_CTRL with at most one
    sync-wait slot; Tile's stock tail attaches all outstanding waits to the
    Drain and fails codegen. Split the waits one-per-NOP instead."""
    nop_inst = self.nc.sync.nop(nofuse=True)
    wait_clock.add_sem_waits(nop_inst.ins, ScopedClock({None: tick_clock.global_clock}))
    si = nop_inst.ins.sync_info
    if si is not None and len(si.on_wait) > 1:
        waits = list(si.on_wait)
        nop_inst.ins.sync_info = mybir.SyncInfo(on_wait=waits[:1], on_update=list(si.on_update))
        for w in waits[1:]:
            n2 = self.nc.sync.nop(nofuse=True)
            n2.ins.sync_info = mybir.SyncInfo(on_wait=[w], on_update=[])
    self.nc.sync.drain()
    self.nc.all_engine_barrier()
    assert self.sems is not None
    popped = self.nc._tile_sem_poison_stack.pop()
    assert popped is self._sem_poison
    self.nc.clear_and_free_semaphores(list(self.sems.allocated().values()))
    self.nc.all_engine_barrier()


tile.TileContext._drain_and_barrier = _drain_and_barrier

_MAX_WAITS = 1  # this walrus build caps sync-waits per instruction


def split_sync_waits(nc):
    """Hoist excess on_wait entries onto same-engine NOPs inserted before the
    instruction (engine queues execute in program order, so semantics hold)."""
    n = 0
    for bb in nc.main_func.blocks:
        insts = bb.instructions
        new_list = []
        for inst in insts:
            si = getattr(inst, "sync_info", None)
            if si is not None and len(si.on_wait) > _MAX_WAITS:
                waits = list(si.on_wait)
                for w in waits[:-_MAX_WAITS]:
                    n += 1
                    new_list.append(mybir.InstNoOp(
                        name=f"{inst.name}-sw{n}",
                        sync_info=mybir.SyncInfo(on_wait=[w], on_update=[]),
                        bass_nofuse=True,
                        engine=inst.engine,
                    ))
                inst.sync_info = mybir.SyncInfo(
                    on_wait=waits[-_MAX_WAITS:], on_update=list(si.on_update)
                )
            new_list.append(inst)
        if len(new_list) != len(insts):
            bb.instructions[:] = new_list
    return n


# Model dims (hardcoded per problem spec)
L_FULL, H, D, V, SMAX = 8, 16, 1024, 32000, 1024
DH = D // H          # 64
FF = 4 * D           # 4096
B, S = 2, 1024
T = S                # tokens per group (one batch element per group)
TP = 4               # tensor-parallel degree within a group
HL = H // TP         # 4 local heads
FFL = FF // TP       # 1024 local FFN cols
VL = V // TP         # 8000 local vocab cols
VLP = 8064           # padded to 63*128
NVM = VLP // 128     # 63 vocab m-tiles
EPS = 1e-5
KT = D // 128        # 8 k-tiles over model dim
KT2 = KT // 2        # 4 fp8 DoubleRow k-pairs
NB = T // 512        # 2 token blocks of 512

WS = 64.0            # fp8 weight prescale
WSI = 1.0 / WS

BF = mybir.dt.bfloat16
F32 = mybir.dt.float32
F8 = mybir.dt.float8e4
AF = mybir.ActivationFunctionType
ALU = mybir.AluOpType
DR = mybir.MatmulPerfMode.DoubleRow

RG = [[0, 1, 2, 3], [4, 5, 6, 7]]

N_LAYERS = int(os.environ.get("BASS_GPT_LAYERS", str(L_FULL)))
SKIP_FINAL = os.environ.get("BASS_GPT_SKIP_FINAL", "0") == "1"

# Set from the actual inputs before build: when LN gains are all-ones /
# biases all-zero (true for this model family), the g/b passes and bias
# adds are dropped and scale-invariance folds the fp8 compensation into
# the residual adds.
FOLD_LN = True
FOLD_BIAS = True


def _r2(ap):
    """[ (kt p) n ] -> [p kt n] view of a DRAM 2-D tensor (p=128)."""
    return ap.rearrange("(kt p) n -> p kt n", p=128)


def _bc(ap, dim, n):
    """Insert a stride-0 broadcast dim of size n at position `dim`."""
    newap = [list(d) for d in ap.ap]
    newap.insert(dim, [0, n])
    return bass.AP(ap.tensor, ap.offset, newap)


def build_program():
    nc = bass.Bass("TRN2")

    # ---- DRAM parameters (per-core shards, host-reshaped) ----
    h0b = nc.declare_dram_parameter("h0b", [D, T], BF, isOutput=False)
    h0b8 = nc.declare_dram_parameter("h0b8", [128, KT, T], F8, isOutput=False)
    wqkv8 = nc.declare_dram_parameter("wqkv8", [N_LAYERS, 128, KT2, 2, 3 * HL * DH], F8, isOutput=False)
    bqkv = nc.declare_dram_parameter("bqkv", [N_LAYERS, 3 * HL * DH], F32, isOutput=False)
    wo8 = nc.declare_dram_parameter("wo8", [N_LAYERS, 128, KT2, 2, D], F8, isOutput=False)
    bo = nc.declare_dram_parameter("bo", [N_LAYERS, D], F32, isOutput=False)
    ln1g = nc.declare_dram_parameter("ln1g", [N_LAYERS, D], F32, isOutput=False)
    ln1b = nc.declare_dram_parameter("ln1b", [N_LAYERS, D], F32, isOutput=False)
    w1r = nc.declare_dram_parameter("w1r", [N_LAYERS, 128, KT, FFL], BF, isOutput=False)
    b1 = nc.declare_dram_parameter("b1", [N_LAYERS, FFL], F32, isOutput=False)
    w2r = nc.declare_dram_parameter("w2r", [N_LAYERS, 128, KT, D], BF, isOutput=False)
    b2q4 = nc.declare_dram_parameter("b2q4", [N_LAYERS, D], F32, isOutput=False)
    ln2g = nc.declare_dram_parameter("ln2g", [N_LAYERS, D], F32, isOutput=False)
    ln2b = nc.declare_dram_parameter("ln2b", [N_LAYERS, D], F32, isOutput=False)
    woutr = nc.declare_dram_parameter("woutr", [NVM, 128, KT, 128], BF, isOutput=False)
    bout = nc.declare_dram_parameter("bout", [VLP], F32, isOutput=False)
    out = nc.declare_dram_parameter("out", [VLP, T], BF, isOutput=True)

    with ExitStack() as ctx:
        tc = ctx.enter_context(tile.TileContext(nc))

        def _scope(name):
            sid, _ = nc.enter_named_scope(name, False)
            return (name, sid)

        def _escope(h):
            nc.leave_named_scope(h[0], h[1], False)

        # ---- outer pools (live whole program) ----
        const = ctx.enter_context(tc.tile_pool(name="const", bufs=1))
        hpool = ctx.enter_context(tc.tile_pool(name="hpool", bufs=1))
        spool = ctx.enter_context(tc.tile_pool(name="spool", bufs=2))
        rpool = ctx.enter_context(tc.tile_pool(name="rpool", bufs=2))
        mm_psum = ctx.enter_context(tc.tile_pool(name="mm_psum", bufs=3, space="PSUM"))
        bc_psum = ctx.enter_context(tc.tile_pool(name="bc_psum", bufs=1, space="PSUM"))
        dram = ctx.enter_context(tc.tile_pool(name="dram", bufs=2, space="DRAM"))

        # ---- constants ----
        ones_k = const.tile([128, 1], BF)       # lhsT for partition-sum (M=1)
        nc.vector.memset(ones_k, 1.0)
        ones128 = const.tile([128, 128], BF)    # lhsT for bcast partition-sum (M=128)
        nc.vector.memset(ones128, 1.0)
        ones_m = const.tile([1, 128], BF)       # lhsT for broadcast (K=1, M=128)
        nc.vector.memset(ones_m, 1.0)
        eps128 = const.tile([128, 1], F32)
        nc.vector.memset(eps128, float(EPS))
        # causal keep-masks: variant j keeps where t1f - t2p - 128*j >= 0
        maskq = const.tile([128, 4, 512], BF)
        nc.gpsimd.memset(maskq, 1.0)
        for j in range(4):
            nc.gpsimd.affine_select(
                out=maskq[:, j, :], in_=maskq[:, j, :],
                compare_op=ALU.is_ge, fill=0.0,
                base=-128 * j, pattern=[[1, 512]], channel_multiplier=-1,
            )

        # ---- persistent activation state ----
        hb = hpool.tile([128, KT, T], BF)       # residual stream (feature-major)
        nc.sync.dma_start(hb, _r2(h0b))
        hb8 = hpool.tile([128, KT, T], F8)      # fp8 copy for DR matmul rhs
        nc.sync.dma_start(hb8, h0b8[:])

        with ExitStack() as lctx:
            apool = lctx.enter_context(tc.tile_pool(name="apool", bufs=1))
            xpool = lctx.enter_context(tc.tile_pool(name="xpool", bufs=1))
            fpool = lctx.enter_context(tc.tile_pool(name="fpool", bufs=1))
            epool = lctx.enter_context(tc.tile_pool(name="epool", bufs=2))
            wq_pool = lctx.enter_context(tc.tile_pool(name="wq_pool", bufs=2))
            wo_pool = lctx.enter_context(tc.tile_pool(name="wo_pool", bufs=2))
            w1_pool = lctx.enter_context(tc.tile_pool(name="w1_pool", bufs=1))
            w2_pool = lctx.enter_context(tc.tile_pool(name="w2_pool", bufs=1))
            agp = lctx.enter_context(tc.tile_pool(name="agp", bufs=2))
            oarp = lctx.enter_context(tc.tile_pool(name="oarp", bufs=1))
            xsqb = lctx.enter_context(tc.tile_pool(name="xsqb", bufs=1))
            bpool = lctx.enter_context(tc.tile_pool(name="bpool", bufs=2))
            stp = lctx.enter_context(tc.tile_pool(name="stp", bufs=1))
            o_psum = lctx.enter_context(tc.tile_pool(name="o_psum", bufs=2, space="PSUM"))
            st_psum = lctx.enter_context(tc.tile_pool(name="st_psum", bufs=1, space="PSUM"))

            qk_sb = apool.tile([128, 2, 2, T], BF)   # [part, q/k, head-pair, t]
            vaug = apool.tile([128, KT, HL, 65], BF)  # token-major V + ones col
            oT8 = apool.tile([128, 2, T], F8)        # attn head outputs (feature-major)
            x1b = xpool.tile([128, KT, T], BF)       # pre-LN1 accumulator
            nc.vector.memset(vaug[:, :, :, 64:65], 1.0)

            def ln_stats(xsrc, tsl, tag):
                """Broadcast stats for tokens tsl of xsrc [128,KT,*] bf16.
                Returns (mu_bc, rp_bc) SBUF [128,512]; rp via exp(-.5 ln).
                Scale-invariant consumers let xsrc carry any uniform scale."""
                ps_s1 = st_psum.tile([128, 512], F32, tag="st1")
                ps_s2 = st_psum.tile([128, 512], F32, tag="st2")
                xsq = xsqb.tile([128, KT, 512], BF, tag="xsq")
                nc.vector.tensor_mul(xsq[:, 0:5, :], xsrc[:, 0:5, tsl], xsrc[:, 0:5, tsl])
                nc.scalar.activation(xsq[:, 5:8, :], xsrc[:, 5:8, tsl], AF.Square)
                for kt in range(KT):
                    nc.tensor.matmul(ps_s1, ones128, xsrc[:, kt, tsl],
                                     start=(kt == 0), stop=(kt == KT - 1))
                for kt in range(KT):
                    nc.tensor.matmul(ps_s2, ones128, xsq[:, kt, :],
                                     start=(kt == 0), stop=(kt == KT - 1))
                mu_bc = stp.tile([128, 512], BF, tag=f"mu{tag}")
                nc.vector.tensor_scalar(out=mu_bc, in0=ps_s1, scalar1=1.0 / D,
                                        scalar2=None, op0=ALU.mult)
                m1sq = spool.tile([128, 512], BF, tag="m1sq")
                nc.vector.tensor_mul(m1sq, mu_bc, mu_bc)
                u = spool.tile([128, 512], BF, tag="uvar")
                nc.vector.scalar_tensor_tensor(
                    out=u, in0=ps_s2, scalar=1.0 / D, in1=m1sq,
                    op0=ALU.mult, op1=ALU.subtract)
                lnu = spool.tile([128, 512], BF, tag="lnu")
                nc.scalar.activation(lnu, u, AF.Ln, bias=eps128[:, 0:1])
                rp_bc = stp.tile([128, 512], BF, tag=f"rp{tag}")
                nc.scalar.activation(rp_bc, lnu, AF.Exp, scale=-0.5)
                return mu_bc, rp_bc

            def ln_apply(xsrc, tsl, mu_bc, rp_bc, g_sb, b_sb, hb_tsl=None):
                """hb[:, :, hb_tsl] = (xsrc - mu)*rp*g + b ; hb8 = fp8(hb).
                Batched over kt with stride-0 broadcast of mu/rp; when
                FOLD_LN (g==1, b==0) the g/b passes are dropped."""
                if hb_tsl is None:
                    hb_tsl = tsl
                xm = xsqb.tile([128, KT, 512], BF, tag="xmb")
                nc.vector.tensor_sub(xm[:, 0:5, :], xsrc[:, 0:5, tsl], _bc(mu_bc, 1, 5))
                nc.vector.tensor_sub(xm[:, 5:8, :], xsrc[:, 5:8, tsl], _bc(mu_bc, 1, 3))
                if FOLD_LN:
                    nc.vector.tensor_mul(hb[:, 0:5, hb_tsl], xm[:, 0:5, :], _bc(rp_bc, 1, 5))
                    nc.vector.tensor_mul(hb[:, 5:8, hb_tsl], xm[:, 5:8, :], _bc(rp_bc, 1, 3))
                else:
                    nc.vector.tensor_mul(xm[:, 0:5, :], xm[:, 0:5, :], _bc(rp_bc, 1, 5))
                    nc.gpsimd.tensor_mul(xm[:, 5:8, :], xm[:, 5:8, :], _bc(rp_bc, 1, 3))
                    g3 = _bc(g_sb[:, 0:KT], 2, 512)
                    b3 = _bc(b_sb[:, 0:KT], 2, 512)
                    nc.vector.tensor_mul(xm, xm, g3)
                    nc.vector.tensor_add(hb[:, :, hb_tsl], xm, b3)
                nc.vector.tensor_copy(hb8[:, 0:5, hb_tsl], hb[:, 0:5, hb_tsl])
                nc.scalar.activation(hb8[:, 5:8, hb_tsl], hb[:, 5:8, hb_tsl], AF.Copy)

            def phase_qkv(l, wqkv_sb, bqkv_sb, blk):
                tsl = slice(blk * 512, (blk + 1) * 512)
                for io in range(2):        # 0=q, 1=k  (feature-major out)
                    for mt in range(2):    # head pair
                        mcol = (io * 2 + mt) * 128
                        ps = mm_psum.tile([128, 512], F32, tag="mm")
                        for j in range(KT2):
                            nc.tensor.matmul(
                                ps, wqkv_sb[:, j, :, mcol:mcol + 128],
                                hb8[:, 2 * j:2 * j + 2, tsl],
                                start=(j == 0), stop=(j == KT2 - 1), perf_mode=DR)
                        nc.vector.tensor_scalar(
                            out=qk_sb[:, io, mt, tsl], in0=ps,
                            scalar1=WSI, scalar2=bqkv_sb[:, io * 2 + mt:io * 2 + mt + 1],
                            op0=ALU.mult, op1=ALU.add)
                for tm in range(4 * blk, 4 * (blk + 1)):   # v, token-major
                    ps = mm_psum.tile([128, 256], F32, tag="mm")
                    for j in range(KT2):
                        nc.tensor.matmul(
                            ps, hb8[:, 2 * j:2 * j + 2, tm * 128:(tm + 1) * 128],
                            wqkv_sb[:, j, :, 512:768],
                            start=(j == 0), stop=(j == KT2 - 1), perf_mode=DR)
                    nc.vector.tensor_scalar(
                        out=vaug[:, tm, :, 0:64],
                        in0=ps.rearrange("p (h e) -> p h e", h=HL),
                        scalar1=WSI, scalar2=None, op0=ALU.mult)

            def phase_attn(l, bqkv_sb, blk):
                t1sl = slice(blk * 512, (blk + 1) * 512)
                t2max = 4 * (blk + 1)
                for h in range(HL):
                    prow = slice(64 * (h % 2), 64 * (h % 2) + 64)
                    hm = h // 2
                    et = epool.tile([128, KT, 512], BF, tag="eT")
                    for t2t in range(t2max):
                        ps = mm_psum.tile([128, 512], F32, tag="mm")
                        nc.tensor.matmul(
                            ps,
                            qk_sb[prow, 1, hm, t2t * 128:(t2t + 1) * 128],
                            qk_sb[prow, 0, hm, t1sl],
                            start=True, stop=True)
                        nc.scalar.activation(et[:, t2t, :], ps, AF.Exp, scale=0.125)
                    dg = slice(4 * blk, 4 * blk + 4)
                    nc.vector.tensor_mul(et[:, dg, :], et[:, dg, :], maskq)
                    ps_o = o_psum.tile([65, 512], F32, tag="o")
                    for t2t in range(t2max):
                        nc.tensor.matmul(
                            ps_o, vaug[:, t2t, h, :], et[:, t2t, :],
                            start=(t2t == 0), stop=(t2t == t2max - 1))
                    # 1/Z via exp(-ln(Z)) on the Activation engine
                    lnz = rpool.tile([1, 512], F32, tag="lnz")
                    nc.scalar.activation(lnz, ps_o[64:65, :], AF.Ln)
                    rec = rpool.tile([1, 512], BF, tag="rec")
                    nc.scalar.activation(rec, lnz, AF.Exp, scale=-1.0)
                    ps_b = bc_psum.tile([64, 512], F32, tag="bc")
                    nc.tensor.matmul(ps_b, ones_m[:, 0:64], rec, start=True, stop=True)
                    psb_sb = spool.tile([64, 512], BF, tag="psb")
                    nc.vector.tensor_copy(psb_sb, ps_b)
                    tmp = spool.tile([64, 512], BF, tag="otmp")
                    nc.vector.tensor_mul(tmp, ps_o[0:64, :], psb_sb)
                    nc.vector.tensor_scalar(
                        out=oT8[prow, hm, t1sl], in0=tmp,
                        scalar1=bqkv_sb[prow, 4 + hm:5 + hm], scalar2=None,
                        op0=ALU.add)

            def stage_ag(blk):
                tsl = slice(blk * 512, (blk + 1) * 512)
                ag_in = dram.tile([2 * 128, 512], F8, tag="agin")
                for pt in range(2):
                    nc.sync.dma_start(ag_in[pt * 128:(pt + 1) * 128, :], oT8[:, pt, tsl])
                ag_out = dram.tile([D, 512], F8, tag="agout")
                nc.gpsimd.collective_compute(
                    "AllGather", ALU.bypass, replica_groups=RG,
                    ins=[ag_in.opt()], outs=[ag_out.opt()])
                return ag_out

            def phase_oproj(l, wo_sb, bo_sb, ag_out, blk):
                """Full out-proj from gathered head outputs; x1b = oproj*WSI + bo + hb."""
                tsl = slice(blk * 512, (blk + 1) * 512)
                agh = agp.tile([128, KT, 512], F8, tag="agh")
                nc.sync.dma_start(agh, _r2(ag_out))
                for mt in range(KT):
                    ps = mm_psum.tile([128, 512], F32, tag="mm")
                    for j in range(KT2):
                        nc.tensor.matmul(
                            ps, wo_sb[:, j, :, mt * 128:(mt + 1) * 128],
                            agh[:, 2 * j:2 * j + 2, :],
                            start=(j == 0), stop=(j == KT2 - 1), perf_mode=DR)
                    if FOLD_BIAS:
                        # x1b = WS*(oproj + hb); LN1 is scale-invariant
                        nc.vector.scalar_tensor_tensor(
                            out=x1b[:, mt, tsl], in0=hb[:, mt, tsl], scalar=WS,
                            in1=ps, op0=ALU.mult, op1=ALU.add)
                    else:
                        op = spool.tile([128, 512], BF, tag="ob")
                        nc.vector.tensor_scalar(
                            out=op, in0=ps, scalar1=WSI, scalar2=bo_sb[:, mt:mt + 1],
                            op0=ALU.mult, op1=ALU.add)
                        if mt % 2 == 0:
                            nc.gpsimd.tensor_add(x1b[:, mt, tsl], op, hb[:, mt, tsl])
                        else:
                            nc.vector.tensor_add(x1b[:, mt, tsl], op, hb[:, mt, tsl])

            def phase_ffn(l, w1_sb, w2_sb, b1_sb, b2q4_sb, blk):
                tsl = slice(blk * 512, (blk + 1) * 512)
                f1 = fpool.tile([128, KT, 512], BF, tag="f1")
                for mt in range(KT):
                    ps = mm_psum.tile([128, 512], F32, tag="mm")
                    for kt in range(KT):
                        nc.tensor.matmul(
                            ps, w1_sb[:, kt, mt * 128:(mt + 1) * 128],
                            hb[:, kt, tsl],
                            start=(kt == 0), stop=(kt == KT - 1))
                    nc.scalar.activation(f1[:, mt, :], ps, AF.Relu,
                                         bias=b1_sb[:, mt:mt + 1])
                ar_in = dram.tile([D, 512], BF, tag="arin")
                for mt in range(KT):
                    ps = mm_psum.tile([128, 512], F32, tag="mm")
                    for kt in range(KT):
                        nc.tensor.matmul(
                            ps, w2_sb[:, kt, mt * 128:(mt + 1) * 128],
                            f1[:, kt, :],
                            start=(kt == 0), stop=(kt == KT - 1))
                    if FOLD_BIAS:
                        # ob = hb/4 + ff_partial -> AR yields x2 exactly
                        ob = spool.tile([128, 512], BF, tag="ob")
                        nc.vector.scalar_tensor_tensor(
                            out=ob, in0=hb[:, mt, tsl], scalar=0.25,
                            in1=ps, op0=ALU.mult, op1=ALU.add)
                    else:
                        hq = spool.tile([128, 512], BF, tag="xm")
                        nc.vector.tensor_scalar(
                            out=hq, in0=hb[:, mt, tsl], scalar1=0.25,
                            scalar2=b2q4_sb[:, mt:mt + 1], op0=ALU.mult, op1=ALU.add)
                        ob = spool.tile([128, 512], BF, tag="ob")
                        nc.vector.scalar_tensor_tensor(
                            out=ob, in0=ps, scalar=WSI, in1=hq,
                            op0=ALU.mult, op1=ALU.add)
                    nc.sync.dma_start(ar_in[mt * 128:(mt + 1) * 128, :], ob)
                ar_out = dram.tile([D, 512], BF, tag="arout")
                nc.gpsimd.collective_compute(
                    "AllReduce", ALU.add, replica_groups=RG,
                    ins=[ar_in.opt()], outs=[ar_out.opt()])
                return ar_out

            def phase_ln2(l, ar_out, g_sb, b_sb, blk):
                """AR output already includes the (scaled) residual; LN2 -> hb."""
                tsl = slice(blk * 512, (blk + 1) * 512)
                oar = oarp.tile([128, KT, 512], BF, tag="oar")
                nc.sync.dma_start(oar, _r2(ar_out))
                mu, rp = ln_stats(oar, slice(0, 512), "b")
                ln_apply(oar, slice(0, 512), mu, rp, g_sb, b_sb, hb_tsl=tsl)

            # ---- layer loop, software-pipelined across the block dim ----
            pend_f1 = None   # deferred (ar_out1, g2_sb, bb2_sb) from prev layer
            for l in range(N_LAYERS):
                _h = _scope(f"L{l}.wload")
                wqkv_sb = wq_pool.tile([128, KT2, 2, 768], F8, tag="wqkv")
                nc.sync.dma_start(wqkv_sb, wqkv8[l])
                wo_sb = wo_pool.tile([128, KT2, 2, D], F8, tag="wo")
                nc.sync.dma_start(wo_sb, wo8[l])
                w1_sb = w1_pool.tile([128, KT, FFL], BF, tag="w1")
                nc.sync.dma_start(w1_sb, w1r[l])
                w2_sb = w2_pool.tile([128, KT, D], BF, tag="w2")
                nc.sync.dma_start(w2_sb, w2r[l])
                bqkv_sb = bpool.tile([128, 6], F32, tag="bqkv")
                nc.sync.dma_start(bqkv_sb, bqkv[l].rearrange("(m p) -> p m", p=128))
                bo_sb = bpool.tile([128, KT], F32, tag="bo")
                nc.sync.dma_start(bo_sb, bo[l].rearrange("(m p) -> p m", p=128))
                g1_sb = bpool.tile([128, KT], F32, tag="g1")
                nc.sync.dma_start(g1_sb, ln1g[l].rearrange("(m p) -> p m", p=128))
                bb1_sb = bpool.tile([128, KT], F32, tag="bb1")
                nc.sync.dma_start(bb1_sb, ln1b[l].rearrange("(m p) -> p m", p=128))
                b1_sb = bpool.tile([128, KT], F32, tag="b1")
                nc.sync.dma_start(b1_sb, b1[l].rearrange("(m p) -> p m", p=128))
                b2q4_sb = bpool.tile([128, KT], F32, tag="b2")
                nc.sync.dma_start(b2q4_sb, b2q4[l].rearrange("(m p) -> p m", p=128))
                g2_sb = bpool.tile([128, KT], F32, tag="g2")
                nc.sync.dma_start(g2_sb, ln2g[l].rearrange("(m p) -> p m", p=128))
                bb2_sb = bpool.tile([128, KT], F32, tag="bb2")
                nc.sync.dma_start(bb2_sb, ln2b[l].rearrange("(m p) -> p m", p=128))
                _escope(_h)

                _h = _scope(f"L{l}.ab0")
                phase_qkv(l, wqkv_sb, bqkv_sb, 0)
                phase_attn(l, bqkv_sb, 0)
                ag0 = stage_ag(0)
                _escope(_h)

                if pend_f1 is not None:
                    _h = _scope(f"L{l}.f1prev")
                    phase_ln2(l - 1, *pend_f1, 1)
                    _escope(_h)
                    pend_f1 = None

                _h = _scope(f"L{l}.ab1")
                phase_qkv(l, wqkv_sb, bqkv_sb, 1)
                phase_attn(l, bqkv_sb, 1)
                ag1 = stage_ag(1)
                _escope(_h)

                _h = _scope(f"L{l}.d0")
                phase_oproj(l, wo_sb, bo_sb, ag0, 0)
                mu0, rp0 = ln_stats(x1b, slice(0, 512), "a")
                _escope(_h)
                _h = _scope(f"L{l}.d1")
                phase_oproj(l, wo_sb, bo_sb, ag1, 1)
                _escope(_h)
                _h = _scope(f"L{l}.e0")
                ln_apply(x1b, slice(0, 512), mu0, rp0, g1_sb, bb1_sb)
                ar0 = phase_ffn(l, w1_sb, w2_sb, b1_sb, b2q4_sb, 0)
                _escope(_h)
                _h = _scope(f"L{l}.e1")
                mu1, rp1 = ln_stats(x1b, slice(512, 1024), "a")
                ln_apply(x1b, slice(512, 1024), mu1, rp1, g1_sb, bb1_sb)
                ar1 = phase_ffn(l, w1_sb, w2_sb, b1_sb, b2q4_sb, 1)
                _escope(_h)
                _h = _scope(f"L{l}.f0")
                phase_ln2(l, ar0, g2_sb, bb2_sb, 0)
                _escope(_h)
                pend_f1 = (ar1, g2_sb, bb2_sb)

            _h = _scope("flast")
            phase_ln2(N_LAYERS - 1, *pend_f1, 1)
            _escope(_h)

        # ---- vocab projection + log-softmax (layer pools now closed) ----
        _h = _scope("vocab")
        if not SKIP_FINAL:
            with ExitStack() as vctx:
                lgp = vctx.enter_context(tc.tile_pool(name="lgp", bufs=2))
                vwch = vctx.enter_context(tc.tile_pool(name="vwch", bufs=6))
                vspool = vctx.enter_context(tc.tile_pool(name="vspool", bufs=2))
                vdram = vctx.enter_context(tc.tile_pool(name="vdram", bufs=2, space="DRAM"))
                va_psum = vctx.enter_context(tc.tile_pool(name="va_psum", bufs=2, space="PSUM"))

                bout_sb = const.tile([128, NVM], F32)
                nc.sync.dma_start(bout_sb, bout.rearrange("(m p) -> p m", p=128))
                for th in range(2):
                    tsl = slice(th * 512, (th + 1) * 512)
                    lg = lgp.tile([128, NVM, 512], BF, tag="lg")
                    ps_acc = va_psum.tile([1, 512], F32, tag="vacc")
                    for vm in range(NVM):
                        wv_sb = vwch.tile([128, KT, 128], BF, tag="vw")
                        nc.sync.dma_start(wv_sb, woutr[vm])
                        ps = mm_psum.tile([128, 512], F32, tag="mm")
                        for kt in range(KT):
                            nc.tensor.matmul(ps, wv_sb[:, kt, :], hb[:, kt, tsl],
                                             start=(kt == 0), stop=(kt == KT - 1))
                        if vm % 2 == 0:
                            nc.vector.tensor_scalar(
                                out=lg[:, vm, :], in0=ps, scalar1=bout_sb[:, vm:vm + 1],
                                scalar2=None, op0=ALU.add)
                        else:
                            nc.scalar.activation(lg[:, vm, :], ps, AF.Identity,
                                                 bias=bout_sb[:, vm:vm + 1])
                        eb = vspool.tile([128, 512], BF, tag="eb")
                        nc.scalar.activation(eb, ps, AF.Exp,
                                             bias=bout_sb[:, vm:vm + 1])
                        nc.tensor.matmul(
                            ps_acc, ones_k, eb,
                            start=(vm == 0), stop=(vm == NVM - 1), skip_group_check=True)
                    se_row = rpool.tile([1, 512], F32, tag="serow")
                    nc.vector.tensor_copy(se_row, ps_acc)
                    se_in = vdram.tile([1, 512], F32, tag="sein")
                    nc.sync.dma_start(se_in, se_row)
                    se_out = vdram.tile([1, 512], F32, tag="seout")
                    nc.gpsimd.collective_compute(
                        "AllReduce", ALU.add, replica_groups=RG,
                        ins=[se_in.opt()], outs=[se_out.opt()])
                    se_sb = rpool.tile([1, 512], F32, tag="sesb")
                    nc.sync.dma_start(se_sb, se_out)
                    lr = rpool.tile([1, 512], BF, tag="lr")
                    nc.scalar.activation(lr, se_sb, AF.Ln)
                    psl = bc_psum.tile([128, 512], F32, tag="bc")
                    nc.tensor.matmul(psl, ones_m, lr, start=True, stop=True)
                    psl_sb = vspool.tile([128, 512], BF, tag="psl")
                    nc.vector.tensor_copy(psl_sb, psl)
                    for gi in range(9):   # 9 groups of 7 vocab m-tiles
                        vs = slice(gi * 7, gi * 7 + 7)
                        outf = vspool.tile([128, 7, 512], BF, tag="outf", bufs=1)
                        nc.vector.tensor_sub(outf, lg[:, vs, :], _bc(psl_sb, 1, 7))
                        nc.sync.dma_start(
                            out[gi * 7 * 128:(gi + 1) * 7 * 128, tsl]
                            .rearrange("(vm p) t -> p vm t", p=128), outf)
        else:
            # debug: dump hb as bf16 into the first D rows of out
            for kt in range(KT):
                dbg = spool.tile([128, T], BF, tag="outf")
                nc.vector.tensor_copy(dbg, hb[:, kt, :])
                nc.sync.dma_start(out[kt * 128:(kt + 1) * 128, :], dbg)
        _escope(_h)

    nsplit = split_sync_waits(nc)
    print(f"split_sync_waits: {nsplit} NOPs inserted")
    return nc


def _bf16(a):
    return np.asarray(a, dtype=ml_dtypes.bfloat16)


def _f8(a):
    return np.asarray(a, dtype=ml_dtypes.float8_e4m3)


def _pack_dr(w):
    """[1024, C] f32 -> [128, KT2, 2, C] fp8 with x64 prescale.
    Row r = 256*j + 128*i + p maps to [p, j, i, c]."""
    r = (np.asarray(w, np.float32) * WS).reshape(KT2, 2, 128, -1).transpose(2, 0, 1, 3)
    return _f8(np.ascontiguousarray(r))


def make_in_maps(x, tok_emb, pos_emb, wq, bq, wk, bk, wv, bv, wo, bo,
                 ln1_g, ln1_b, w1, b1, w2, b2, ln2_g, ln2_b, w_out, b_out):
    """Shard full inputs -> per-core input maps (host-reshaped fp8 weights)."""
    LE = wq.shape[0]
    per_r = []
    for r in range(TP):
        hs = slice(HL * r, HL * (r + 1))
        wqkv_r = np.concatenate(
            [
                wq[:, hs].transpose(0, 2, 1, 3).reshape(LE, D, HL * DH),
                wk[:, hs].transpose(0, 2, 1, 3).reshape(LE, D, HL * DH),
                wv[:, hs].transpose(0, 2, 1, 3).reshape(LE, D, HL * DH),
            ],
            axis=2,
        )
        bqkv_r = np.concatenate(
            [bq[:, hs].reshape(LE, -1), bk[:, hs].reshape(LE, -1),
             bv[:, hs].reshape(LE, -1)], axis=1,
        )
        fs = slice(FFL * r, FFL * (r + 1))
        vs = slice(VL * r, VL * (r + 1))
        wqkv8_r = np.stack([_pack_dr(wqkv_r[le]) for le in range(LE)])
        wo8_r = np.stack([_pack_dr(wo[le]) for le in range(LE)])
        w1r_r = np.stack([_bf16(np.ascontiguousarray(
            np.asarray(w1[le][:, fs], np.float32).reshape(KT, 128, FFL)
            .transpose(1, 0, 2))) for le in range(LE)])
        w2r_r = np.stack([_bf16(np.ascontiguousarray(
            np.asarray(w2[le][fs, :], np.float32).reshape(KT, 128, D)
            .transpose(1, 0, 2))) for le in range(LE)])
        wout_r = np.zeros((D, VLP), np.float32)
        wout_r[:, :VL] = w_out[:, vs]
        woutr_r = _bf16(np.ascontiguousarray(
            np.transpose(wout_r.reshape(KT, 128, NVM, 128), (2, 1, 0, 3))))
        bout_r = np.full((VLP,), -1e30, np.float32)
        bout_r[:VL] = b_out[vs]
        per_r.append(dict(
            wqkv8=wqkv8_r,
            bqkv=np.ascontiguousarray(bqkv_r, np.float32),
            wo8=wo8_r,
            bo=np.ascontiguousarray(bo, np.float32),
            ln1g=np.ascontiguousarray(ln1_g, np.float32),
            ln1b=np.ascontiguousarray(ln1_b, np.float32),
            w1r=w1r_r,
            b1=np.ascontiguousarray(b1[:, fs], np.float32),
            w2r=w2r_r,
            b2q4=np.ascontiguousarray(b2 / TP, np.float32),
            ln2g=np.ascontiguousarray(ln2_g, np.float32),
            ln2b=np.ascontiguousarray(ln2_b, np.float32),
            woutr=woutr_r,
            bout=bout_r,
        ))
    in_maps = []
    for c in range(8):
        g, r = c // TP, c % TP
        emb = np.asarray(tok_emb[x[g]] + pos_emb[:S], np.float32)   # [S, D]
        m = dict(per_r[r])
        embT = np.ascontiguousarray(emb.T)                          # [D, T]
        m["h0b"] = _bf16(embT)
        m["h0b8"] = _f8(np.ascontiguousarray(
            embT.reshape(KT, 128, T).transpose(1, 0, 2)))
        in_maps.append(m)
    return in_maps


_CACHED = {}


def kernel(**inputs):
    global FOLD_LN, FOLD_BIAS
    inputs = {k: np.asarray(v) for k, v in inputs.items()}
    fold_ln = (np.all(inputs["ln1_g"] == 1) and np.all(inputs["ln1_b"] == 0)
               and np.all(inputs["ln2_g"] == 1) and np.all(inputs["ln2_b"] == 0))
    fold_bias = (np.all(inputs["bo"] == 0) and np.all(inputs["b2"] == 0))
    key = ("nc", fold_ln, fold_bias)
    if key not in _CACHED:
        FOLD_LN, FOLD_BIAS = fold_ln, fold_bias
        _CACHED[key] = build_program()
    nc = _CACHED[key]
    in_maps = make_in_maps(**inputs)
    trace = os.environ.get("BASS_GPT_TRACE", "0") == "1"
    res = run_bass_kernel_spmd(
        nc, in_maps, core_ids=list(range(8)), trace=trace,
    )
    if trace:
        print(f"HW exec time: {res.exec_time_ns} ns")
        _CACHED["last_result"] = res
    results = res.results
    full = np.empty((B, S, V), np.float32)
    for c in range(8):
        g, r = c // TP, c % TP
        full[g, :, VL * r: VL * (r + 1)] = \
            np.asarray(results[c]["out"][:VL, :], np.float32).T
    return full
